# revision 38
# baseline (speedup 1.0000x reference)
"""AspectAttention Trainium2 kernel (8 NeuronCores, pure data parallel).

out[b, n] = sum_e softmax_n(tanh(h @ W_a + b_a))[b, n, e] * h[b, n, e]

Self-contained: hardcodes shapes B=4096, N=64, D=256, 8 cores.

Per-core dataflow (512 batches), batch-on-partitions layout:
  - SWDGE cast-DMA on the sync queue: h f32 DRAM -> bf16 SBUF chunks
    [128 b, 64 n, 256 d] (contiguous per partition -> full-rate)
  - PE transpose per (n, d-half): [128 b, 128 d] -> PSUM [128 d, 128 b] bf16
  - ACT copies PSUM -> SBUF hT slabs as packed uint32 (half the elems),
    queued ahead of exp so the PE never waits on hT
  - PE matmul: stationary hT-slab [d, b], moving W [d, e] -> PSUM s[b, e],
    accumulated over both d-halves
  - ACT: custom LUT where `Exp` evaluates exp(tanh(x)) (PSUM->SBUF bf16);
    tanh in [-1,1] bounds the softmax domain, so no max-subtraction.
  - Pool: tree-sum over n -> S[b, e] (f32); DVE reciprocal -> R, cast Rb
  - DVE: P1 = E*h in place; F = P1*Rb (R broadcast over n, split DVE/Pool)
  - Sigma_e: in-place halving tree over e (bf16 2x TT, split DVE/Pool)
    then one tensor_reduce(axis=X) on the last 32 -> out[b, n] f32
"""
import hashlib
import json
import os
import shutil
from contextlib import ExitStack

import numpy as np

_PWP_SRC = (
    "/nix/store/z022hj2nvbm3nwdizlisq4ylc0y7rd6q-python3-3.13.14-env/"
    "lib/python3.13/site-packages/neuronxcc/pwp/pwp_bin_trainium"
)


def _exptanh_derivs(x):
    u = np.tanh(x)
    s = 1.0 - u * u
    f = np.exp(u)
    return (f, f * s, f * (s * s - 2 * u * s),
            f * (s**3 - 6 * u * s * s - 2 * s * s + 4 * u * u * s))


def _install_act_tables():
    """Build ACT tables where func `exp` evaluates exp(tanh(x)). Returns a
    content hash so the compile cache keys on the table contents."""
    global _PWP_SRC
    if not os.path.isdir(_PWP_SRC):
        from neuronxcc.driver.Job import Job
        from neuronxcc.driver.jobs.support.FindActInfo import findActInfoFile
        _PWP_SRC = os.path.dirname(findActInfoFile(Job.getPackageDir(), "gen3"))
    dst = "/tmp/aspect_act_tables_v1"
    if not os.path.exists(os.path.join(dst, "act_info.json")):
        tmp = dst + ".tmp"
        if os.path.exists(tmp):
            shutil.rmtree(tmp)
        shutil.copytree(_PWP_SRC, tmp)
        bkt_path = os.path.join(tmp, "exp_and_others_bkt.bin")
        b = np.fromfile(bkt_path, dtype=np.float32).reshape(-1, 8).copy()
        x0 = b[:, 4].astype(np.float64)
        d0, d1, d2 = b[:, 0], b[:, 1], b[:, 2]
        with np.errstate(over="ignore", invalid="ignore"):
            ex = np.exp(np.clip(x0, -87.0, 87.0))
            is_exp = (np.isfinite(d0)
                      & (np.abs(d0 - ex) <= 1e-3 * np.maximum(ex, 1e-30))
                      & (np.abs(d1 - d0) <= 1e-3 * np.abs(d0) + 1e-30)
                      & (np.abs(d2 - d0 / 2) <= 1e-3 * np.abs(d0) + 1e-30))
        idx = np.where(is_exp)[0]
        f, f1, f2, f3 = _exptanh_derivs(x0[idx])
        b[idx, 0] = f.astype(np.float32)
        b[idx, 1] = f1.astype(np.float32)
        b[idx, 2] = (f2 / 2.0).astype(np.float32)
        b[idx, 3] = (f3 / 6.0).astype(np.float32)
        b[779] = [np.float32(np.e), 0, 0, 0, 0, 0, 0, 0]
        b[780] = [np.float32(1 / np.e), 0, 0, 0, 0, 0, 0, 0]
        b.tofile(bkt_path)
        pj_path = os.path.join(tmp, "exp_and_others.json")
        pj = json.load(open(pj_path))
        for fm in pj["profile_meta_data"]:
            if fm["func_name"].startswith("exp"):
                fm["fpinf_result"] = int(np.float32(np.e).view(np.uint32))
                fm["fninf_result"] = int(np.float32(1 / np.e).view(np.uint32))
        json.dump(pj, open(pj_path, "w"))
        os.replace(tmp, dst) if not os.path.exists(dst) else None
    os.environ["BASS_ACT_ROOT_JSON_PATH"] = os.path.join(dst, "act_info.json")
    hsh = hashlib.sha256(
        open(os.path.join(dst, "exp_and_others_bkt.bin"), "rb").read()
    ).hexdigest()[:8]
    return hsh

import concourse.bass as bass
import concourse.tile as tile
from concourse import bacc, mybir
from concourse.bass_utils import run_bass_kernel_spmd

N_CORES = 8
B_FULL, N_BLOCK, D = 4096, 64, 256
B_SHARD = B_FULL // N_CORES  # 512
P = 128
N_CHUNKS = B_SHARD // P  # 4
F32 = mybir.dt.float32
BF16 = mybir.dt.bfloat16
U32 = mybir.dt.uint32
ALU = mybir.AluOpType
ACT_T = mybir.ActivationFunctionType


def _quarter_tree(eng, E, tr_p, s_p, q):
    """Sum E[:, q*16:(q+1)*16, :] over n with a TT add tree on `eng`.
    Quarter granularity starts the DVE two n-groups earlier than halves and
    shortens the post-exp tail chain."""
    a = q * 16
    t1 = tr_p.tile([P, 8, D], BF16, tag=f"tq1_{q}")
    t2 = tr_p.tile([P, 4, D], BF16, tag=f"tq2_{q}")
    eng.tensor_tensor(t1[:], E[:, a:a + 8, :], E[:, a + 8:a + 16, :], ALU.add)
    eng.tensor_tensor(t2[:], t1[:, 0:4, :], t1[:, 4:8, :], ALU.add)
    eng.tensor_tensor(t1[:, 0:2, :], t2[:, 0:2, :], t2[:, 2:4, :], ALU.add)
    sq = s_p.tile([P, D], F32, tag=f"Sq{q}")
    eng.tensor_tensor(sq[:], t1[:, 0, :], t1[:, 1, :], ALU.add)
    return sq


def _half_tree(eng, E, tr_p, s_p, half):
    """Sum E[:, half*32:(half+1)*32, :] over n with a TT add tree on `eng`.
    Returns the [P, D] f32 partial sum tile."""
    n0 = half * 32
    tr1 = tr_p.tile([P, 16, D], BF16, tag=f"tr1_{half}")
    tr2 = tr_p.tile([P, 8, D], BF16, tag=f"tr2_{half}")
    eng.tensor_tensor(tr1[:], E[:, n0:n0 + 16, :], E[:, n0 + 16:n0 + 32, :],
                      ALU.add)
    eng.tensor_tensor(tr2[:], tr1[:, 0:8, :], tr1[:, 8:16, :], ALU.add)
    eng.tensor_tensor(tr1[:, 0:4, :], tr2[:, 0:4, :], tr2[:, 4:8, :], ALU.add)
    eng.tensor_tensor(tr2[:, 0:2, :], tr1[:, 0:2, :], tr1[:, 2:4, :], ALU.add)
    sh = s_p.tile([P, D], F32, tag=f"Sh{half}")
    eng.tensor_tensor(sh[:], tr2[:, 0, :], tr2[:, 1, :], ALU.add)
    return sh


def build_fast(tbl_hash):
    """Optimized no-bias path."""
    nc = bacc.Bacc("TRN2", debug=False, num_devices=N_CORES)
    tbl_d = nc.dram_tensor(f"tblkey_{tbl_hash}", [1, 4], F32, kind="ExternalInput")
    h_d = nc.dram_tensor("h", [B_SHARD, N_BLOCK, D], F32, kind="ExternalInput")
    w_d = nc.dram_tensor("W_a", [D, D], F32, kind="ExternalInput")
    ident_d = nc.dram_tensor("ident", [P, P], BF16, kind="ExternalInput")
    ones_d = nc.dram_tensor("ones", [P, 1], BF16, kind="ExternalInput")
    out_d = nc.dram_tensor("out", [B_SHARD, N_BLOCK], F32, kind="ExternalOutput")

    with tile.TileContext(nc) as tc, ExitStack() as ctx:
        const_p = ctx.enter_context(tc.tile_pool(name="const", bufs=1))
        h_p = ctx.enter_context(tc.tile_pool(name="h", bufs=2))
        e_p = ctx.enter_context(tc.tile_pool(name="E", bufs=2))
        ht_p = ctx.enter_context(tc.tile_pool(name="hT", bufs=2))
        tr_p = ctx.enter_context(tc.tile_pool(name="tr", bufs=1))
        s_p = ctx.enter_context(tc.tile_pool(name="S", bufs=2))
        o_p = ctx.enter_context(tc.tile_pool(name="o", bufs=2))
        scr_p = ctx.enter_context(tc.tile_pool(name="scr", bufs=2))
        psT_p = ctx.enter_context(tc.tile_pool(name="psT", bufs=2, space="PSUM"))
        psS_p = ctx.enter_context(tc.tile_pool(name="psS", bufs=2, space="PSUM"))
        psO_p = ctx.enter_context(tc.tile_pool(name="psO", bufs=2, space="PSUM"))

        tblk = const_p.tile([1, 4], F32)
        nc.sync.dma_start(tblk[:], tbl_d.ap())
        ident = const_p.tile([P, P], BF16)
        nc.sync.dma_start(ident[:], ident_d.ap())
        ones = const_p.tile([P, 1], BF16)
        nc.sync.dma_start(ones[:], ones_d.ap())
        wf = const_p.tile([P, 2, D], F32)
        nc.sync.dma_start(wf[:, 0, :], w_d.ap()[0:P, :])
        nc.sync.dma_start(wf[:, 1, :], w_d.ap()[P:2 * P, :])
        wb = const_p.tile([P, 2, D], BF16)
        nc.vector.tensor_copy(wb[:], wf[:])

        def issue_load(c):
            bs = c * P
            t = h_p.tile([P, N_BLOCK, D], BF16)
            # chunk 0's first quarter arrives as 4n slivers so the PE can
            # start transposing ~5us earlier during the pipeline fill
            gsz = 4 if c == 0 else 16
            for g in range(N_BLOCK // gsz):
                nc.gpsimd.dma_start(
                    t[:, g * gsz:(g + 1) * gsz, :],
                    h_d.ap()[bs:bs + P, g * gsz:(g + 1) * gsz, :],
                )
                if c == 0 and g == 3:
                    gsz = 16
                    # remaining 48 n in three 16n loads
                    for g2 in range(1, 4):
                        nc.gpsimd.dma_start(
                            t[:, g2 * 16:(g2 + 1) * 16, :],
                            h_d.ap()[bs:bs + P, g2 * 16:(g2 + 1) * 16, :],
                        )
                    break
            return t

        def emit_sigma_ng(prev, ngF):
            """Sigma_e over e for n-group ngF of the PREVIOUS chunk: PE
            re-transposes F per (n, e-half), a copy moves the slab to SBUF,
            then per n two 1-moving-row matmuls contract the 128 e-partitions
            against the ones vector into psO[:, n]."""
            E_prev, psO = prev["E"], prev["psO"]
            FT = ht_p.tile([P, 16, P], BF16, tag="", name="FT")
            for half in range(2):
                psFT = psT_p.tile([P, 8, P], BF16, tag="psT", name="psFT")
                for j in range(4):
                    n = ngF * 8 + half * 4 + j
                    for eh in range(2):
                        nc.tensor.transpose(
                            psFT[:, 2 * j + eh, :],
                            E_prev[:, n, eh * P:(eh + 1) * P],
                            ident[:],
                        )
                dst = FT[:, half * 8:half * 8 + 8, :].bitcast(F32)
                if ngF % 2 == 0:
                    nc.scalar.copy(dst, psFT[:].bitcast(F32))
                else:
                    nc.vector.tensor_copy(dst, psFT[:].bitcast(F32))
            for j in range(8):
                n = ngF * 8 + j
                nc.tensor.matmul(
                    psO[:, n:n + 1], FT[:, 2 * j, :], ones[:],
                    start=True, stop=False)
                nc.tensor.matmul(
                    psO[:, n:n + 1], FT[:, 2 * j + 1, :], ones[:],
                    start=False, stop=True)

        def finish_prev(prev):
            staged = o_p.tile([P, N_BLOCK], F32, tag="staged")
            nc.scalar.copy(staged[:], prev["psO"][:])
            nc.sync.dma_start(
                out_d.ap()[prev["bs"]:prev["bs"] + P, :], staged[:])

        def emit_act_reduce(prev, k):
            """Final Sigma_e for n 36+7k..36+7k+6 of the PREVIOUS chunk on
            the ACT engine: Copy with accum_out sums the 256 e-elements per
            partition. ACT coexists with DVE at full rate (unlike Pool), and
            emitting inside the next chunk's ng loop avoids head-of-line
            blocking the ACT queue behind the previous chunk's DVE tail."""
            E_prev, staged_prev = prev["E"], prev["staged"]
            scr = scr_p.tile([P, D], BF16, name="scr")
            n0 = prev["act_n0"] + 7 * k
            for n in range(n0, min(n0 + 7, N_BLOCK)):
                nc.scalar.activation(scr[:], E_prev[:, n, :], ACT_T.Copy,
                                     accum_out=staged_prev[:, n:n + 1])

        prev = None
        h_tiles = {0: issue_load(0)}
        for c in range(N_CHUNKS):
            bs = c * P
            # pre-issue next chunk's load ahead of this chunk's Pool work
            if c + 1 < N_CHUNKS:
                h_tiles[c + 1] = issue_load(c + 1)
            h_nat = h_tiles.pop(c)
            E = e_p.tile([P, N_BLOCK, D], BF16)
            sqs = []  # per-quarter n-tree partial sums
            psS_prev = None  # (psS tile, n0) pending exp
            for ng in range(8):  # n-groups of 8
                hT = ht_p.tile([P, 16, P], BF16)
                for half in range(2):
                    psT = psT_p.tile([P, 8, P], BF16, tag="psT")
                    for j in range(4):
                        n = ng * 8 + half * 4 + j
                        for dh in range(2):
                            nc.tensor.transpose(
                                psT[:, 2 * j + dh, :],
                                h_nat[:, n, dh * P:(dh + 1) * P],
                                ident[:],
                            )
                    # packed f32-bitcast copy: half the elements of a bf16
                    # copy (f32->f32 Copy is a bit-exact identity for
                    # normals; bf16 pairs never form denormal/NaN f32
                    # patterns since the high bf16 is a finite normal)
                    nc.scalar.copy(
                        hT[:, half * 8:half * 8 + 8, :].bitcast(F32),
                        psT[:].bitcast(F32))
                # lagged exp from the previous n-group keeps the ACT queue
                # ordered copy(ng) -> exp(ng-1,q1) so PE never waits on hT
                if psS_prev is not None:
                    ps, pn0 = psS_prev
                    nc.scalar.activation(E[:, pn0:pn0 + 4, :], ps[:], ACT_T.Exp)
                for q in range(2):
                    psS = psS_p.tile([P, 4, D], F32)
                    for j in range(4):
                        jj = q * 4 + j
                        nc.tensor.matmul(
                            psS[:, j, :], hT[:, 2 * jj, :], wb[:, 0, :],
                            start=True, stop=False,
                        )
                        nc.tensor.matmul(
                            psS[:, j, :], hT[:, 2 * jj + 1, :], wb[:, 1, :],
                            start=False, stop=True,
                        )
                    if q == 0:
                        nc.scalar.activation(
                            E[:, ng * 8:ng * 8 + 4, :], psS[:], ACT_T.Exp)
                    else:
                        psS_prev = (psS, ng * 8 + 4)
                if prev is not None and 3 <= ng <= 6:
                    emit_act_reduce(prev, ng - 3)
                if ng == 4:
                    # half 0 of E is complete: start its n-tree on DVE and
                    # the in-place P1 = E*h (after the tree's only E-read,
                    # level 1)
                    sh0 = _half_tree(nc.vector, E, tr_p, s_p, 0)
                    nc.vector.tensor_tensor(
                        E[:, 0:32, :], E[:, 0:32, :], h_nat[:, 0:32, :],
                        ALU.mult)
            ps, pn0 = psS_prev
            nc.scalar.activation(E[:, pn0:pn0 + 4, :], ps[:], ACT_T.Exp)
            if prev is not None:
                nc.sync.dma_start(
                    out_d.ap()[prev["bs"]:prev["bs"] + P, :],
                    prev["staged"][:])
            sh1 = _half_tree(nc.vector, E, tr_p, s_p, 1)
            nc.vector.tensor_tensor(
                E[:, 32:64, :], E[:, 32:64, :], h_nat[:, 32:64, :], ALU.mult)

            S = s_p.tile([P, D], F32, tag="S")
            nc.vector.tensor_tensor(S[:], sh0[:], sh1[:], ALU.add)
            R = s_p.tile([P, D], F32, tag="R")
            nc.vector.reciprocal_approx_fast(R[:], S[:])
            Rb = s_p.tile([P, D], BF16, tag="Rb")
            nc.vector.tensor_copy(Rb[:], R[:])

            # F = P1 * R (R broadcast over n) in place on DVE. Half-size
            # instructions keep the 2x bf16 mode (the full-size variant
            # measured at ~1x).
            rb_b = Rb[:].unsqueeze(1).broadcast_to((P, 32, D))
            staged = o_p.tile([P, N_BLOCK], F32, tag="staged")
            # Sigma_e: full-width tensor_reduce per n-half on DVE, right
            # after that half's F so the first reduce overlaps the second
            # multiply. Pool stays off the hot path entirely (concurrent
            # Pool streaming inflates DVE instruction time ~2.5x via SBUF
            # contention) and the PE-based reduction variants lose to this
            # on measured wall-clock.
            nc.vector.tensor_tensor(E[:, 0:32, :], E[:, 0:32, :], rb_b,
                                    ALU.mult)
            nc.vector.tensor_reduce(
                staged[:, 0:32], E[:, 0:32, :], mybir.AxisListType.X,
                ALU.add)
            nc.vector.tensor_tensor(E[:, 32:64, :], E[:, 32:64, :], rb_b,
                                    ALU.mult)
            act_n0 = 40
            nc.vector.tensor_reduce(
                staged[:, 32:act_n0], E[:, 32:act_n0, :],
                mybir.AxisListType.X, ALU.add)
            prev = {"E": E, "staged": staged, "bs": bs, "act_n0": act_n0}
        # epilogue: the last chunk's ACT reduces run concurrently with its
        # DVE tensor_reduce tail, then the store
        for k in range(4):
            emit_act_reduce(prev, k)
        nc.sync.dma_start(out_d.ap()[prev["bs"]:prev["bs"] + P, :],
                          prev["staged"][:])
    nc.compile()
    return nc


def build_bias(tbl_hash):
    """Bias path (kept from the known-good baseline; b_a is zeros in the
    reference setup so this is correctness insurance only)."""
    nc = bacc.Bacc("TRN2", debug=False, num_devices=N_CORES)
    tbl_d = nc.dram_tensor(f"tblkey_{tbl_hash}", [1, 4], F32, kind="ExternalInput")
    h_d = nc.dram_tensor("h", [B_SHARD, N_BLOCK, D], F32, kind="ExternalInput")
    w_d = nc.dram_tensor("W_a", [D, D], F32, kind="ExternalInput")
    ident_d = nc.dram_tensor("ident", [P, P], BF16, kind="ExternalInput")
    ba_d = nc.dram_tensor("b_a", [N_BLOCK, D], F32, kind="ExternalInput")
    out_d = nc.dram_tensor("out", [B_SHARD, N_BLOCK], F32, kind="ExternalOutput")

    with tile.TileContext(nc) as tc, ExitStack() as ctx:
        const_p = ctx.enter_context(tc.tile_pool(name="const", bufs=1))
        h_p = ctx.enter_context(tc.tile_pool(name="h", bufs=2))
        e_p = ctx.enter_context(tc.tile_pool(name="E", bufs=2))
        ht_p = ctx.enter_context(tc.tile_pool(name="hT", bufs=2))
        t_p = ctx.enter_context(tc.tile_pool(name="t", bufs=2))
        tr_p = ctx.enter_context(tc.tile_pool(name="tr", bufs=1))
        s_p = ctx.enter_context(tc.tile_pool(name="S", bufs=2))
        scr_p = ctx.enter_context(tc.tile_pool(name="scr", bufs=4))
        o_p = ctx.enter_context(tc.tile_pool(name="o", bufs=2))
        psT_p = ctx.enter_context(tc.tile_pool(name="psT", bufs=2, space="PSUM"))
        psS_p = ctx.enter_context(tc.tile_pool(name="psS", bufs=2, space="PSUM"))

        tblk = const_p.tile([1, 4], F32)
        nc.sync.dma_start(tblk[:], tbl_d.ap())
        ident = const_p.tile([P, P], BF16)
        nc.sync.dma_start(ident[:], ident_d.ap())
        wf = const_p.tile([P, 2, D], F32)
        nc.sync.dma_start(wf[:, 0, :], w_d.ap()[0:P, :])
        nc.sync.dma_start(wf[:, 1, :], w_d.ap()[P:2 * P, :])
        wb = const_p.tile([P, 2, D], BF16)
        nc.vector.tensor_copy(wb[:], wf[:])
        bab = const_p.tile([P, N_BLOCK, D], BF16)
        src = ba_d.ap().rearrange("(one n) d -> one n d", one=1)
        src = src.broadcast_to((P, N_BLOCK, D))
        nc.gpsimd.dma_start(bab[:], src)

        for c in range(N_CHUNKS):
            bs = c * P
            h_nat = h_p.tile([P, N_BLOCK, D], BF16)
            for g in range(4):
                nc.gpsimd.dma_start(
                    h_nat[:, g * 16:(g + 1) * 16, :],
                    h_d.ap()[bs:bs + P, g * 16:(g + 1) * 16, :],
                )
            E = e_p.tile([P, N_BLOCK, D], BF16)
            for ng in range(8):
                psT = psT_p.tile([P, 16, P], BF16)
                hT = ht_p.tile([P, 16, P], BF16)
                for j in range(8):
                    n = ng * 8 + j
                    for dh in range(2):
                        nc.tensor.transpose(
                            psT[:, 2 * j + dh, :],
                            h_nat[:, n, dh * P:(dh + 1) * P],
                            ident[:],
                        )
                nc.scalar.copy(hT[:], psT[:])
                for q in range(2):
                    psS = psS_p.tile([P, 4, D], F32)
                    for j in range(4):
                        jj = q * 4 + j
                        nc.tensor.matmul(
                            psS[:, j, :], hT[:, 2 * jj, :], wb[:, 0, :],
                            start=True, stop=False,
                        )
                        nc.tensor.matmul(
                            psS[:, j, :], hT[:, 2 * jj + 1, :], wb[:, 1, :],
                            start=False, stop=True,
                        )
                    n0 = ng * 8 + q * 4
                    tb = t_p.tile([P, 4, D], F32, tag="tbias")
                    nc.vector.tensor_add(tb[:], psS[:], bab[:, n0:n0 + 4, :])
                    nc.scalar.activation(E[:, n0:n0 + 4, :], tb[:], ACT_T.Exp)

            s_half = []
            for half in range(2):
                n0 = half * 32
                tr1 = tr_p.tile([P, 16, D], BF16, tag=f"tr1_{half}")
                tr2 = tr_p.tile([P, 8, D], BF16, tag=f"tr2_{half}")
                nc.vector.tensor_tensor(
                    tr1[:], E[:, n0:n0 + 16, :], E[:, n0 + 16:n0 + 32, :], ALU.add)
                nc.vector.tensor_tensor(
                    tr2[:], tr1[:, 0:8, :], tr1[:, 8:16, :], ALU.add)
                nc.vector.tensor_tensor(
                    tr1[:, 0:4, :], tr2[:, 0:4, :], tr2[:, 4:8, :], ALU.add)
                nc.vector.tensor_tensor(
                    tr2[:, 0:2, :], tr1[:, 0:2, :], tr1[:, 2:4, :], ALU.add)
                sh = s_p.tile([P, D], F32, tag=f"Sh{half}")
                nc.vector.tensor_tensor(sh[:], tr2[:, 0, :], tr2[:, 1, :], ALU.add)
                s_half.append(sh)
                nc.vector.tensor_tensor(
                    E[:, n0:n0 + 32, :], E[:, n0:n0 + 32, :],
                    h_nat[:, n0:n0 + 32, :], ALU.mult
                )
            S = s_p.tile([P, D], F32, tag="S")
            nc.vector.tensor_tensor(S[:], s_half[0][:], s_half[1][:], ALU.add)
            R = s_p.tile([P, D], F32, tag="R")
            nc.vector.reciprocal(R[:], S[:])
            Rb = s_p.tile([P, D], BF16, tag="Rb")
            nc.vector.tensor_copy(Rb[:], R[:])

            out_sb = o_p.tile([P, N_BLOCK], F32, tag="out_sb")
            for n in range(N_BLOCK):
                scr = scr_p.tile([P, D], BF16)
                nc.vector.scalar_tensor_tensor(
                    out=scr[:],
                    in0=E[:, n, :],
                    scalar=1.0,
                    in1=Rb[:],
                    op0=ALU.mult,
                    op1=ALU.mult,
                    accum_out=out_sb[:, n:n + 1],
                )
            staged = o_p.tile([P, N_BLOCK], F32, tag="staged")
            nc.vector.tensor_copy(staged[:], out_sb[:])
            nc.gpsimd.dma_start(out_d.ap()[bs:bs + P, :], staged[:])
    nc.compile()
    return nc


_CACHE = {}


def _get_nc(with_bias: bool):
    if with_bias not in _CACHE:
        tbl_hash = _install_act_tables()
        _CACHE[with_bias] = (
            build_bias(tbl_hash) if with_bias else build_fast(tbl_hash))
    return _CACHE[with_bias]


def run(h, W_a, b_a, trace=False):
    import ml_dtypes

    tbl_hash = _install_act_tables()

    h = np.ascontiguousarray(np.asarray(h, dtype=np.float32))
    W_a = np.ascontiguousarray(np.asarray(W_a, dtype=np.float32))
    b_a = np.ascontiguousarray(np.asarray(b_a, dtype=np.float32))
    with_bias = bool(np.any(b_a))
    nc = _get_nc(with_bias)
    ident = np.eye(P, dtype=ml_dtypes.bfloat16)
    in_maps = []
    for i in range(N_CORES):
        m = {
            "h": h[i * B_SHARD: (i + 1) * B_SHARD],
            "W_a": W_a,
            "ident": ident,
            f"tblkey_{tbl_hash}": np.zeros((1, 4), np.float32),
        }
        if with_bias:
            m["b_a"] = b_a
        else:
            m["ones"] = np.ones((P, 1), dtype=ml_dtypes.bfloat16)
        in_maps.append(m)
    res = run_bass_kernel_spmd(nc, in_maps, core_ids=list(range(N_CORES)), trace=trace)
    out = np.concatenate([res.results[i]["out"] for i in range(N_CORES)], axis=0)
    return out, res


def kernel(h, W_a, b_a):
    out, _ = run(h, W_a, b_a, trace=False)
    return out


# revision 39
# speedup vs baseline: 1.1547x; 1.1547x over previous
"""AspectAttention Trainium2 kernel (8 NeuronCores, pure data parallel).

out[b, n] = sum_e softmax_n(tanh(h @ W_a + b_a))[b, n, e] * h[b, n, e]

Self-contained: hardcodes shapes B=4096, N=64, D=256, 8 cores.

Per-core dataflow (512 batches), batch-on-partitions layout:
  - SWDGE cast-DMA on the sync queue: h f32 DRAM -> bf16 SBUF chunks
    [128 b, 64 n, 256 d] (contiguous per partition -> full-rate)
  - PE transpose per (n, d-half): [128 b, 128 d] -> PSUM [128 d, 128 b] bf16
  - ACT copies PSUM -> SBUF hT slabs as packed uint32 (half the elems),
    queued ahead of exp so the PE never waits on hT
  - PE matmul: stationary hT-slab [d, b], moving W [d, e] -> PSUM s[b, e],
    accumulated over both d-halves
  - ACT: custom LUT where `Exp` evaluates exp(tanh(x)) (PSUM->SBUF bf16);
    tanh in [-1,1] bounds the softmax domain, so no max-subtraction.
  - Pool: tree-sum over n -> S[b, e] (f32); DVE reciprocal -> R, cast Rb
  - DVE: P1 = E*h in place; F = P1*Rb (R broadcast over n, split DVE/Pool)
  - Sigma_e: in-place halving tree over e (bf16 2x TT, split DVE/Pool)
    then one tensor_reduce(axis=X) on the last 32 -> out[b, n] f32
"""
import hashlib
import json
import os
import shutil
from contextlib import ExitStack

import numpy as np

_PWP_SRC = (
    "/nix/store/z022hj2nvbm3nwdizlisq4ylc0y7rd6q-python3-3.13.14-env/"
    "lib/python3.13/site-packages/neuronxcc/pwp/pwp_bin_trainium"
)


def _exptanh_derivs(x):
    u = np.tanh(x)
    s = 1.0 - u * u
    f = np.exp(u)
    return (f, f * s, f * (s * s - 2 * u * s),
            f * (s**3 - 6 * u * s * s - 2 * s * s + 4 * u * u * s))


def _install_act_tables():
    """Build ACT tables where func `exp` evaluates exp(tanh(x)). Returns a
    content hash so the compile cache keys on the table contents."""
    global _PWP_SRC
    if not os.path.isdir(_PWP_SRC):
        from neuronxcc.driver.Job import Job
        from neuronxcc.driver.jobs.support.FindActInfo import findActInfoFile
        _PWP_SRC = os.path.dirname(findActInfoFile(Job.getPackageDir(), "gen3"))
    dst = "/tmp/aspect_act_tables_v1"
    if not os.path.exists(os.path.join(dst, "act_info.json")):
        tmp = dst + ".tmp"
        if os.path.exists(tmp):
            shutil.rmtree(tmp)
        shutil.copytree(_PWP_SRC, tmp)
        bkt_path = os.path.join(tmp, "exp_and_others_bkt.bin")
        b = np.fromfile(bkt_path, dtype=np.float32).reshape(-1, 8).copy()
        x0 = b[:, 4].astype(np.float64)
        d0, d1, d2 = b[:, 0], b[:, 1], b[:, 2]
        with np.errstate(over="ignore", invalid="ignore"):
            ex = np.exp(np.clip(x0, -87.0, 87.0))
            is_exp = (np.isfinite(d0)
                      & (np.abs(d0 - ex) <= 1e-3 * np.maximum(ex, 1e-30))
                      & (np.abs(d1 - d0) <= 1e-3 * np.abs(d0) + 1e-30)
                      & (np.abs(d2 - d0 / 2) <= 1e-3 * np.abs(d0) + 1e-30))
        idx = np.where(is_exp)[0]
        f, f1, f2, f3 = _exptanh_derivs(x0[idx])
        b[idx, 0] = f.astype(np.float32)
        b[idx, 1] = f1.astype(np.float32)
        b[idx, 2] = (f2 / 2.0).astype(np.float32)
        b[idx, 3] = (f3 / 6.0).astype(np.float32)
        b[779] = [np.float32(np.e), 0, 0, 0, 0, 0, 0, 0]
        b[780] = [np.float32(1 / np.e), 0, 0, 0, 0, 0, 0, 0]
        b.tofile(bkt_path)
        pj_path = os.path.join(tmp, "exp_and_others.json")
        pj = json.load(open(pj_path))
        for fm in pj["profile_meta_data"]:
            if fm["func_name"].startswith("exp"):
                fm["fpinf_result"] = int(np.float32(np.e).view(np.uint32))
                fm["fninf_result"] = int(np.float32(1 / np.e).view(np.uint32))
        json.dump(pj, open(pj_path, "w"))
        os.replace(tmp, dst) if not os.path.exists(dst) else None
    os.environ["BASS_ACT_ROOT_JSON_PATH"] = os.path.join(dst, "act_info.json")
    hsh = hashlib.sha256(
        open(os.path.join(dst, "exp_and_others_bkt.bin"), "rb").read()
    ).hexdigest()[:8]
    return hsh

import concourse.bass as bass
import concourse.tile as tile
from concourse import bacc, mybir
from concourse.bass_utils import run_bass_kernel_spmd

N_CORES = 8
B_FULL, N_BLOCK, D = 4096, 64, 256
B_SHARD = B_FULL // N_CORES  # 512
P = 128
N_CHUNKS = B_SHARD // P  # 4
F32 = mybir.dt.float32
BF16 = mybir.dt.bfloat16
U32 = mybir.dt.uint32
ALU = mybir.AluOpType
ACT_T = mybir.ActivationFunctionType


def _quarter_tree(eng, E, tr_p, s_p, q):
    """Sum E[:, q*16:(q+1)*16, :] over n with a TT add tree on `eng`.
    Quarter granularity starts the DVE two n-groups earlier than halves and
    shortens the post-exp tail chain."""
    a = q * 16
    t1 = tr_p.tile([P, 8, D], BF16, tag=f"tq1_{q}")
    t2 = tr_p.tile([P, 4, D], BF16, tag=f"tq2_{q}")
    eng.tensor_tensor(t1[:], E[:, a:a + 8, :], E[:, a + 8:a + 16, :], ALU.add)
    eng.tensor_tensor(t2[:], t1[:, 0:4, :], t1[:, 4:8, :], ALU.add)
    eng.tensor_tensor(t1[:, 0:2, :], t2[:, 0:2, :], t2[:, 2:4, :], ALU.add)
    sq = s_p.tile([P, D], F32, tag=f"Sq{q}")
    eng.tensor_tensor(sq[:], t1[:, 0, :], t1[:, 1, :], ALU.add)
    return sq


def _half_tree(eng, E, tr_p, s_p, half):
    """Sum E[:, half*32:(half+1)*32, :] over n with a TT add tree on `eng`.
    Returns the [P, D] f32 partial sum tile."""
    n0 = half * 32
    tr1 = tr_p.tile([P, 16, D], BF16, tag=f"tr1_{half}")
    tr2 = tr_p.tile([P, 8, D], BF16, tag=f"tr2_{half}")
    eng.tensor_tensor(tr1[:], E[:, n0:n0 + 16, :], E[:, n0 + 16:n0 + 32, :],
                      ALU.add)
    eng.tensor_tensor(tr2[:], tr1[:, 0:8, :], tr1[:, 8:16, :], ALU.add)
    eng.tensor_tensor(tr1[:, 0:4, :], tr2[:, 0:4, :], tr2[:, 4:8, :], ALU.add)
    eng.tensor_tensor(tr2[:, 0:2, :], tr1[:, 0:2, :], tr1[:, 2:4, :], ALU.add)
    sh = s_p.tile([P, D], F32, tag=f"Sh{half}")
    eng.tensor_tensor(sh[:], tr2[:, 0, :], tr2[:, 1, :], ALU.add)
    return sh


def build_fast(tbl_hash):
    """Optimized no-bias path."""
    nc = bacc.Bacc("TRN2", debug=False, num_devices=N_CORES)
    tbl_d = nc.dram_tensor(f"tblkey_{tbl_hash}", [1, 4], F32, kind="ExternalInput")
    h_d = nc.dram_tensor("h", [B_SHARD, N_BLOCK, D], F32, kind="ExternalInput")
    w_d = nc.dram_tensor("W_a", [D, D], F32, kind="ExternalInput")
    ident_d = nc.dram_tensor("ident", [P, P], BF16, kind="ExternalInput")
    ones_d = nc.dram_tensor("ones", [P, 1], BF16, kind="ExternalInput")
    out_d = nc.dram_tensor("out", [B_SHARD, N_BLOCK], F32, kind="ExternalOutput")

    with tile.TileContext(nc) as tc, ExitStack() as ctx:
        const_p = ctx.enter_context(tc.tile_pool(name="const", bufs=1))
        h_p = ctx.enter_context(tc.tile_pool(name="h", bufs=2))
        e_p = ctx.enter_context(tc.tile_pool(name="E", bufs=2))
        ht_p = ctx.enter_context(tc.tile_pool(name="hT", bufs=2))
        tr_p = ctx.enter_context(tc.tile_pool(name="tr", bufs=1))
        s_p = ctx.enter_context(tc.tile_pool(name="S", bufs=2))
        o_p = ctx.enter_context(tc.tile_pool(name="o", bufs=2))
        scr_p = ctx.enter_context(tc.tile_pool(name="scr", bufs=2))
        psT_p = ctx.enter_context(tc.tile_pool(name="psT", bufs=2, space="PSUM"))
        psS_p = ctx.enter_context(tc.tile_pool(name="psS", bufs=2, space="PSUM"))
        psO_p = ctx.enter_context(tc.tile_pool(name="psO", bufs=2, space="PSUM"))

        tblk = const_p.tile([1, 4], F32)
        nc.sync.dma_start(tblk[:], tbl_d.ap())
        ident = const_p.tile([P, P], BF16)
        nc.sync.dma_start(ident[:], ident_d.ap())
        ones = const_p.tile([P, 1], BF16)
        nc.sync.dma_start(ones[:], ones_d.ap())
        wf = const_p.tile([P, 2, D], F32)
        nc.sync.dma_start(wf[:, 0, :], w_d.ap()[0:P, :])
        nc.sync.dma_start(wf[:, 1, :], w_d.ap()[P:2 * P, :])
        wb = const_p.tile([P, 2, D], BF16)
        nc.vector.tensor_copy(wb[:], wf[:])

        def issue_load(c):
            bs = c * P
            t = h_p.tile([P, N_BLOCK, D], BF16)
            # chunk 0's first quarter arrives as 4n slivers so the PE can
            # start transposing ~5us earlier during the pipeline fill
            gsz = 4 if c == 0 else 16
            for g in range(N_BLOCK // gsz):
                nc.gpsimd.dma_start(
                    t[:, g * gsz:(g + 1) * gsz, :],
                    h_d.ap()[bs:bs + P, g * gsz:(g + 1) * gsz, :],
                )
                if c == 0 and g == 3:
                    gsz = 16
                    # remaining 48 n in three 16n loads
                    for g2 in range(1, 4):
                        nc.gpsimd.dma_start(
                            t[:, g2 * 16:(g2 + 1) * 16, :],
                            h_d.ap()[bs:bs + P, g2 * 16:(g2 + 1) * 16, :],
                        )
                    break
            return t

        def emit_sigma_ng(prev, ngF):
            """Sigma_e over e for n-group ngF of the PREVIOUS chunk: PE
            re-transposes F per (n, e-half), a copy moves the slab to SBUF,
            then per n two 1-moving-row matmuls contract the 128 e-partitions
            against the ones vector into psO[:, n]."""
            E_prev, psO = prev["E"], prev["psO"]
            FT = ht_p.tile([P, 16, P], BF16, tag="", name="FT")
            for half in range(2):
                psFT = psT_p.tile([P, 8, P], BF16, tag="psT", name="psFT")
                for j in range(4):
                    n = ngF * 8 + half * 4 + j
                    for eh in range(2):
                        nc.tensor.transpose(
                            psFT[:, 2 * j + eh, :],
                            E_prev[:, n, eh * P:(eh + 1) * P],
                            ident[:],
                        )
                dst = FT[:, half * 8:half * 8 + 8, :].bitcast(F32)
                if ngF % 2 == 0:
                    nc.scalar.copy(dst, psFT[:].bitcast(F32))
                else:
                    nc.vector.tensor_copy(dst, psFT[:].bitcast(F32))
            for j in range(8):
                n = ngF * 8 + j
                nc.tensor.matmul(
                    psO[:, n:n + 1], FT[:, 2 * j, :], ones[:],
                    start=True, stop=False)
                nc.tensor.matmul(
                    psO[:, n:n + 1], FT[:, 2 * j + 1, :], ones[:],
                    start=False, stop=True)

        def finish_prev(prev):
            staged = o_p.tile([P, N_BLOCK], F32, tag="staged")
            nc.scalar.copy(staged[:], prev["psO"][:])
            nc.sync.dma_start(
                out_d.ap()[prev["bs"]:prev["bs"] + P, :], staged[:])

        def emit_act_reduce(prev, k):
            """Final Sigma_e for n 36+7k..36+7k+6 of the PREVIOUS chunk on
            the ACT engine: Copy with accum_out sums the 256 e-elements per
            partition. ACT coexists with DVE at full rate (unlike Pool), and
            emitting inside the next chunk's ng loop avoids head-of-line
            blocking the ACT queue behind the previous chunk's DVE tail."""
            E_prev, staged_prev = prev["E"], prev["staged"]
            scr = scr_p.tile([P, D], BF16, name="scr")
            n0 = prev["act_n0"] + 7 * k
            for n in range(n0, min(n0 + 7, N_BLOCK)):
                nc.scalar.activation(scr[:], E_prev[:, n, :], ACT_T.Copy,
                                     accum_out=staged_prev[:, n:n + 1])

        prev = None
        h_tiles = {0: issue_load(0)}
        for c in range(N_CHUNKS):
            bs = c * P
            # pre-issue next chunk's load ahead of this chunk's Pool work
            if c + 1 < N_CHUNKS:
                h_tiles[c + 1] = issue_load(c + 1)
            h_nat = h_tiles.pop(c)
            E = e_p.tile([P, N_BLOCK, D], BF16)
            sqs = []  # per-quarter n-tree partial sums
            psS_prev = None  # (psS tile, n0) pending exp
            for ng in range(8):  # n-groups of 8
                hT = ht_p.tile([P, 16, P], BF16)
                for half in range(2):
                    psT = psT_p.tile([P, 8, P], BF16, tag="psT")
                    for j in range(4):
                        n = ng * 8 + half * 4 + j
                        for dh in range(2):
                            nc.tensor.transpose(
                                psT[:, 2 * j + dh, :],
                                h_nat[:, n, dh * P:(dh + 1) * P],
                                ident[:],
                            )
                    # packed f32-bitcast copy: half the elements of a bf16
                    # copy (f32->f32 Copy is a bit-exact identity for
                    # normals; bf16 pairs never form denormal/NaN f32
                    # patterns since the high bf16 is a finite normal)
                    nc.scalar.copy(
                        hT[:, half * 8:half * 8 + 8, :].bitcast(F32),
                        psT[:].bitcast(F32))
                # lagged exp from the previous n-group keeps the ACT queue
                # ordered copy(ng) -> exp(ng-1,q1) so PE never waits on hT
                if psS_prev is not None:
                    ps, pn0 = psS_prev
                    nc.scalar.activation(E[:, pn0:pn0 + 4, :], ps[:], ACT_T.Exp)
                for q in range(2):
                    psS = psS_p.tile([P, 4, D], F32)
                    for j in range(4):
                        jj = q * 4 + j
                        nc.tensor.matmul(
                            psS[:, j, :], hT[:, 2 * jj, :], wb[:, 0, :],
                            start=True, stop=False,
                        )
                        nc.tensor.matmul(
                            psS[:, j, :], hT[:, 2 * jj + 1, :], wb[:, 1, :],
                            start=False, stop=True,
                        )
                    if q == 0:
                        nc.scalar.activation(
                            E[:, ng * 8:ng * 8 + 4, :], psS[:], ACT_T.Exp)
                    else:
                        psS_prev = (psS, ng * 8 + 4)
                if prev is not None and 3 <= ng <= 6:
                    emit_act_reduce(prev, ng - 3)
                if ng == 4:
                    # half 0 of E is complete: start its n-tree on DVE and
                    # the in-place P1 = E*h (after the tree's only E-read,
                    # level 1)
                    sh0 = _half_tree(nc.vector, E, tr_p, s_p, 0)
                    nc.vector.tensor_tensor(
                        E[:, 0:32, :], E[:, 0:32, :], h_nat[:, 0:32, :],
                        ALU.mult)
            ps, pn0 = psS_prev
            nc.scalar.activation(E[:, pn0:pn0 + 4, :], ps[:], ACT_T.Exp)
            if prev is not None:
                nc.sync.dma_start(
                    out_d.ap()[prev["bs"]:prev["bs"] + P, :],
                    prev["staged"][:])
            sh1 = _half_tree(nc.vector, E, tr_p, s_p, 1)
            nc.vector.tensor_tensor(
                E[:, 32:64, :], E[:, 32:64, :], h_nat[:, 32:64, :], ALU.mult)

            S = s_p.tile([P, D], F32, tag="S")
            nc.vector.tensor_tensor(S[:], sh0[:], sh1[:], ALU.add)
            R = s_p.tile([P, D], F32, tag="R")
            nc.vector.reciprocal_approx_fast(R[:], S[:])
            Rb = s_p.tile([P, D], BF16, tag="Rb")
            nc.vector.tensor_copy(Rb[:], R[:])

            # F = P1 * R (R broadcast over n) in place on DVE. Half-size
            # instructions keep the 2x bf16 mode (the full-size variant
            # measured at ~1x).
            rb_b = Rb[:].unsqueeze(1).broadcast_to((P, 32, D))
            staged = o_p.tile([P, N_BLOCK], F32, tag="staged")
            # Sigma_e: full-width tensor_reduce per n-half on DVE, right
            # after that half's F so the first reduce overlaps the second
            # multiply. Pool stays off the hot path entirely (concurrent
            # Pool streaming inflates DVE instruction time ~2.5x via SBUF
            # contention) and the PE-based reduction variants lose to this
            # on measured wall-clock.
            nc.vector.tensor_tensor(E[:, 0:32, :], E[:, 0:32, :], rb_b,
                                    ALU.mult)
            nc.vector.tensor_reduce(
                staged[:, 0:32], E[:, 0:32, :], mybir.AxisListType.X,
                ALU.add)
            nc.vector.tensor_tensor(E[:, 32:64, :], E[:, 32:64, :], rb_b,
                                    ALU.mult)
            act_n0 = 64
            nc.vector.tensor_reduce(
                staged[:, 32:act_n0], E[:, 32:act_n0, :],
                mybir.AxisListType.X, ALU.add)
            prev = {"E": E, "staged": staged, "bs": bs, "act_n0": act_n0}
        # epilogue: the last chunk's ACT reduces run concurrently with its
        # DVE tensor_reduce tail, then the store
        for k in range(4):
            emit_act_reduce(prev, k)
        nc.sync.dma_start(out_d.ap()[prev["bs"]:prev["bs"] + P, :],
                          prev["staged"][:])
    nc.compile()
    return nc


def build_bias(tbl_hash):
    """Bias path (kept from the known-good baseline; b_a is zeros in the
    reference setup so this is correctness insurance only)."""
    nc = bacc.Bacc("TRN2", debug=False, num_devices=N_CORES)
    tbl_d = nc.dram_tensor(f"tblkey_{tbl_hash}", [1, 4], F32, kind="ExternalInput")
    h_d = nc.dram_tensor("h", [B_SHARD, N_BLOCK, D], F32, kind="ExternalInput")
    w_d = nc.dram_tensor("W_a", [D, D], F32, kind="ExternalInput")
    ident_d = nc.dram_tensor("ident", [P, P], BF16, kind="ExternalInput")
    ba_d = nc.dram_tensor("b_a", [N_BLOCK, D], F32, kind="ExternalInput")
    out_d = nc.dram_tensor("out", [B_SHARD, N_BLOCK], F32, kind="ExternalOutput")

    with tile.TileContext(nc) as tc, ExitStack() as ctx:
        const_p = ctx.enter_context(tc.tile_pool(name="const", bufs=1))
        h_p = ctx.enter_context(tc.tile_pool(name="h", bufs=2))
        e_p = ctx.enter_context(tc.tile_pool(name="E", bufs=2))
        ht_p = ctx.enter_context(tc.tile_pool(name="hT", bufs=2))
        t_p = ctx.enter_context(tc.tile_pool(name="t", bufs=2))
        tr_p = ctx.enter_context(tc.tile_pool(name="tr", bufs=1))
        s_p = ctx.enter_context(tc.tile_pool(name="S", bufs=2))
        scr_p = ctx.enter_context(tc.tile_pool(name="scr", bufs=4))
        o_p = ctx.enter_context(tc.tile_pool(name="o", bufs=2))
        psT_p = ctx.enter_context(tc.tile_pool(name="psT", bufs=2, space="PSUM"))
        psS_p = ctx.enter_context(tc.tile_pool(name="psS", bufs=2, space="PSUM"))

        tblk = const_p.tile([1, 4], F32)
        nc.sync.dma_start(tblk[:], tbl_d.ap())
        ident = const_p.tile([P, P], BF16)
        nc.sync.dma_start(ident[:], ident_d.ap())
        wf = const_p.tile([P, 2, D], F32)
        nc.sync.dma_start(wf[:, 0, :], w_d.ap()[0:P, :])
        nc.sync.dma_start(wf[:, 1, :], w_d.ap()[P:2 * P, :])
        wb = const_p.tile([P, 2, D], BF16)
        nc.vector.tensor_copy(wb[:], wf[:])
        bab = const_p.tile([P, N_BLOCK, D], BF16)
        src = ba_d.ap().rearrange("(one n) d -> one n d", one=1)
        src = src.broadcast_to((P, N_BLOCK, D))
        nc.gpsimd.dma_start(bab[:], src)

        for c in range(N_CHUNKS):
            bs = c * P
            h_nat = h_p.tile([P, N_BLOCK, D], BF16)
            for g in range(4):
                nc.gpsimd.dma_start(
                    h_nat[:, g * 16:(g + 1) * 16, :],
                    h_d.ap()[bs:bs + P, g * 16:(g + 1) * 16, :],
                )
            E = e_p.tile([P, N_BLOCK, D], BF16)
            for ng in range(8):
                psT = psT_p.tile([P, 16, P], BF16)
                hT = ht_p.tile([P, 16, P], BF16)
                for j in range(8):
                    n = ng * 8 + j
                    for dh in range(2):
                        nc.tensor.transpose(
                            psT[:, 2 * j + dh, :],
                            h_nat[:, n, dh * P:(dh + 1) * P],
                            ident[:],
                        )
                nc.scalar.copy(hT[:], psT[:])
                for q in range(2):
                    psS = psS_p.tile([P, 4, D], F32)
                    for j in range(4):
                        jj = q * 4 + j
                        nc.tensor.matmul(
                            psS[:, j, :], hT[:, 2 * jj, :], wb[:, 0, :],
                            start=True, stop=False,
                        )
                        nc.tensor.matmul(
                            psS[:, j, :], hT[:, 2 * jj + 1, :], wb[:, 1, :],
                            start=False, stop=True,
                        )
                    n0 = ng * 8 + q * 4
                    tb = t_p.tile([P, 4, D], F32, tag="tbias")
                    nc.vector.tensor_add(tb[:], psS[:], bab[:, n0:n0 + 4, :])
                    nc.scalar.activation(E[:, n0:n0 + 4, :], tb[:], ACT_T.Exp)

            s_half = []
            for half in range(2):
                n0 = half * 32
                tr1 = tr_p.tile([P, 16, D], BF16, tag=f"tr1_{half}")
                tr2 = tr_p.tile([P, 8, D], BF16, tag=f"tr2_{half}")
                nc.vector.tensor_tensor(
                    tr1[:], E[:, n0:n0 + 16, :], E[:, n0 + 16:n0 + 32, :], ALU.add)
                nc.vector.tensor_tensor(
                    tr2[:], tr1[:, 0:8, :], tr1[:, 8:16, :], ALU.add)
                nc.vector.tensor_tensor(
                    tr1[:, 0:4, :], tr2[:, 0:4, :], tr2[:, 4:8, :], ALU.add)
                nc.vector.tensor_tensor(
                    tr2[:, 0:2, :], tr1[:, 0:2, :], tr1[:, 2:4, :], ALU.add)
                sh = s_p.tile([P, D], F32, tag=f"Sh{half}")
                nc.vector.tensor_tensor(sh[:], tr2[:, 0, :], tr2[:, 1, :], ALU.add)
                s_half.append(sh)
                nc.vector.tensor_tensor(
                    E[:, n0:n0 + 32, :], E[:, n0:n0 + 32, :],
                    h_nat[:, n0:n0 + 32, :], ALU.mult
                )
            S = s_p.tile([P, D], F32, tag="S")
            nc.vector.tensor_tensor(S[:], s_half[0][:], s_half[1][:], ALU.add)
            R = s_p.tile([P, D], F32, tag="R")
            nc.vector.reciprocal(R[:], S[:])
            Rb = s_p.tile([P, D], BF16, tag="Rb")
            nc.vector.tensor_copy(Rb[:], R[:])

            out_sb = o_p.tile([P, N_BLOCK], F32, tag="out_sb")
            for n in range(N_BLOCK):
                scr = scr_p.tile([P, D], BF16)
                nc.vector.scalar_tensor_tensor(
                    out=scr[:],
                    in0=E[:, n, :],
                    scalar=1.0,
                    in1=Rb[:],
                    op0=ALU.mult,
                    op1=ALU.mult,
                    accum_out=out_sb[:, n:n + 1],
                )
            staged = o_p.tile([P, N_BLOCK], F32, tag="staged")
            nc.vector.tensor_copy(staged[:], out_sb[:])
            nc.gpsimd.dma_start(out_d.ap()[bs:bs + P, :], staged[:])
    nc.compile()
    return nc


_CACHE = {}


def _get_nc(with_bias: bool):
    if with_bias not in _CACHE:
        tbl_hash = _install_act_tables()
        _CACHE[with_bias] = (
            build_bias(tbl_hash) if with_bias else build_fast(tbl_hash))
    return _CACHE[with_bias]


def run(h, W_a, b_a, trace=False):
    import ml_dtypes

    tbl_hash = _install_act_tables()

    h = np.ascontiguousarray(np.asarray(h, dtype=np.float32))
    W_a = np.ascontiguousarray(np.asarray(W_a, dtype=np.float32))
    b_a = np.ascontiguousarray(np.asarray(b_a, dtype=np.float32))
    with_bias = bool(np.any(b_a))
    nc = _get_nc(with_bias)
    ident = np.eye(P, dtype=ml_dtypes.bfloat16)
    in_maps = []
    for i in range(N_CORES):
        m = {
            "h": h[i * B_SHARD: (i + 1) * B_SHARD],
            "W_a": W_a,
            "ident": ident,
            f"tblkey_{tbl_hash}": np.zeros((1, 4), np.float32),
        }
        if with_bias:
            m["b_a"] = b_a
        else:
            m["ones"] = np.ones((P, 1), dtype=ml_dtypes.bfloat16)
        in_maps.append(m)
    res = run_bass_kernel_spmd(nc, in_maps, core_ids=list(range(N_CORES)), trace=trace)
    out = np.concatenate([res.results[i]["out"] for i in range(N_CORES)], axis=0)
    return out, res


def kernel(h, W_a, b_a):
    out, _ = run(h, W_a, b_a, trace=False)
    return out


# revision 40
# speedup vs baseline: 1.1820x; 1.0237x over previous
"""AspectAttention Trainium2 kernel (8 NeuronCores, pure data parallel).

out[b, n] = sum_e softmax_n(tanh(h @ W_a + b_a))[b, n, e] * h[b, n, e]

Self-contained: hardcodes shapes B=4096, N=64, D=256, 8 cores.

Per-core dataflow (512 batches), batch-on-partitions layout:
  - SWDGE cast-DMA on the sync queue: h f32 DRAM -> bf16 SBUF chunks
    [128 b, 64 n, 256 d] (contiguous per partition -> full-rate)
  - PE transpose per (n, d-half): [128 b, 128 d] -> PSUM [128 d, 128 b] bf16
  - ACT copies PSUM -> SBUF hT slabs as packed uint32 (half the elems),
    queued ahead of exp so the PE never waits on hT
  - PE matmul: stationary hT-slab [d, b], moving W [d, e] -> PSUM s[b, e],
    accumulated over both d-halves
  - ACT: custom LUT where `Exp` evaluates exp(tanh(x)) (PSUM->SBUF bf16);
    tanh in [-1,1] bounds the softmax domain, so no max-subtraction.
  - Pool: tree-sum over n -> S[b, e] (f32); DVE reciprocal -> R, cast Rb
  - DVE: P1 = E*h in place; F = P1*Rb (R broadcast over n, split DVE/Pool)
  - Sigma_e: in-place halving tree over e (bf16 2x TT, split DVE/Pool)
    then one tensor_reduce(axis=X) on the last 32 -> out[b, n] f32
"""
import hashlib
import json
import os
import shutil
from contextlib import ExitStack

import numpy as np

_PWP_SRC = (
    "/nix/store/z022hj2nvbm3nwdizlisq4ylc0y7rd6q-python3-3.13.14-env/"
    "lib/python3.13/site-packages/neuronxcc/pwp/pwp_bin_trainium"
)


def _exptanh_derivs(x):
    u = np.tanh(x)
    s = 1.0 - u * u
    f = np.exp(u)
    return (f, f * s, f * (s * s - 2 * u * s),
            f * (s**3 - 6 * u * s * s - 2 * s * s + 4 * u * u * s))


def _install_act_tables():
    """Build ACT tables where func `exp` evaluates exp(tanh(x)). Returns a
    content hash so the compile cache keys on the table contents."""
    global _PWP_SRC
    if not os.path.isdir(_PWP_SRC):
        from neuronxcc.driver.Job import Job
        from neuronxcc.driver.jobs.support.FindActInfo import findActInfoFile
        _PWP_SRC = os.path.dirname(findActInfoFile(Job.getPackageDir(), "gen3"))
    dst = "/tmp/aspect_act_tables_v1"
    if not os.path.exists(os.path.join(dst, "act_info.json")):
        tmp = dst + ".tmp"
        if os.path.exists(tmp):
            shutil.rmtree(tmp)
        shutil.copytree(_PWP_SRC, tmp)
        bkt_path = os.path.join(tmp, "exp_and_others_bkt.bin")
        b = np.fromfile(bkt_path, dtype=np.float32).reshape(-1, 8).copy()
        x0 = b[:, 4].astype(np.float64)
        d0, d1, d2 = b[:, 0], b[:, 1], b[:, 2]
        with np.errstate(over="ignore", invalid="ignore"):
            ex = np.exp(np.clip(x0, -87.0, 87.0))
            is_exp = (np.isfinite(d0)
                      & (np.abs(d0 - ex) <= 1e-3 * np.maximum(ex, 1e-30))
                      & (np.abs(d1 - d0) <= 1e-3 * np.abs(d0) + 1e-30)
                      & (np.abs(d2 - d0 / 2) <= 1e-3 * np.abs(d0) + 1e-30))
        idx = np.where(is_exp)[0]
        f, f1, f2, f3 = _exptanh_derivs(x0[idx])
        b[idx, 0] = f.astype(np.float32)
        b[idx, 1] = f1.astype(np.float32)
        b[idx, 2] = (f2 / 2.0).astype(np.float32)
        b[idx, 3] = (f3 / 6.0).astype(np.float32)
        b[779] = [np.float32(np.e), 0, 0, 0, 0, 0, 0, 0]
        b[780] = [np.float32(1 / np.e), 0, 0, 0, 0, 0, 0, 0]
        b.tofile(bkt_path)
        pj_path = os.path.join(tmp, "exp_and_others.json")
        pj = json.load(open(pj_path))
        for fm in pj["profile_meta_data"]:
            if fm["func_name"].startswith("exp"):
                fm["fpinf_result"] = int(np.float32(np.e).view(np.uint32))
                fm["fninf_result"] = int(np.float32(1 / np.e).view(np.uint32))
        json.dump(pj, open(pj_path, "w"))
        os.replace(tmp, dst) if not os.path.exists(dst) else None
    os.environ["BASS_ACT_ROOT_JSON_PATH"] = os.path.join(dst, "act_info.json")
    hsh = hashlib.sha256(
        open(os.path.join(dst, "exp_and_others_bkt.bin"), "rb").read()
    ).hexdigest()[:8]
    return hsh

import concourse.bass as bass
import concourse.tile as tile
from concourse import bacc, mybir
from concourse.bass_utils import run_bass_kernel_spmd

N_CORES = 8
B_FULL, N_BLOCK, D = 4096, 64, 256
B_SHARD = B_FULL // N_CORES  # 512
P = 128
N_CHUNKS = B_SHARD // P  # 4
F32 = mybir.dt.float32
BF16 = mybir.dt.bfloat16
U32 = mybir.dt.uint32
ALU = mybir.AluOpType
ACT_T = mybir.ActivationFunctionType


def _quarter_tree(eng, E, tr_p, s_p, q):
    """Sum E[:, q*16:(q+1)*16, :] over n with a TT add tree on `eng`.
    Quarter granularity starts the DVE two n-groups earlier than halves and
    shortens the post-exp tail chain."""
    a = q * 16
    t1 = tr_p.tile([P, 8, D], BF16, tag=f"tq1_{q}")
    t2 = tr_p.tile([P, 4, D], BF16, tag=f"tq2_{q}")
    eng.tensor_tensor(t1[:], E[:, a:a + 8, :], E[:, a + 8:a + 16, :], ALU.add)
    eng.tensor_tensor(t2[:], t1[:, 0:4, :], t1[:, 4:8, :], ALU.add)
    eng.tensor_tensor(t1[:, 0:2, :], t2[:, 0:2, :], t2[:, 2:4, :], ALU.add)
    sq = s_p.tile([P, D], F32, tag=f"Sq{q}")
    eng.tensor_tensor(sq[:], t1[:, 0, :], t1[:, 1, :], ALU.add)
    return sq


def _half_tree(eng, E, tr_p, s_p, half):
    """Sum E[:, half*32:(half+1)*32, :] over n with a TT add tree on `eng`.
    Returns the [P, D] f32 partial sum tile."""
    n0 = half * 32
    tr1 = tr_p.tile([P, 16, D], BF16, tag=f"tr1_{half}")
    tr2 = tr_p.tile([P, 8, D], BF16, tag=f"tr2_{half}")
    eng.tensor_tensor(tr1[:], E[:, n0:n0 + 16, :], E[:, n0 + 16:n0 + 32, :],
                      ALU.add)
    eng.tensor_tensor(tr2[:], tr1[:, 0:8, :], tr1[:, 8:16, :], ALU.add)
    eng.tensor_tensor(tr1[:, 0:4, :], tr2[:, 0:4, :], tr2[:, 4:8, :], ALU.add)
    eng.tensor_tensor(tr2[:, 0:2, :], tr1[:, 0:2, :], tr1[:, 2:4, :], ALU.add)
    sh = s_p.tile([P, D], F32, tag=f"Sh{half}")
    eng.tensor_tensor(sh[:], tr2[:, 0, :], tr2[:, 1, :], ALU.add)
    return sh


def build_fast(tbl_hash):
    """Optimized no-bias path."""
    nc = bacc.Bacc("TRN2", debug=False, num_devices=N_CORES)
    tbl_d = nc.dram_tensor(f"tblkey_{tbl_hash}", [1, 4], F32, kind="ExternalInput")
    h_d = nc.dram_tensor("h", [B_SHARD, N_BLOCK, D], F32, kind="ExternalInput")
    w_d = nc.dram_tensor("W_a", [D, D], F32, kind="ExternalInput")
    ident_d = nc.dram_tensor("ident", [P, P], BF16, kind="ExternalInput")
    ones_d = nc.dram_tensor("ones", [P, 1], BF16, kind="ExternalInput")
    out_d = nc.dram_tensor("out", [B_SHARD, N_BLOCK], F32, kind="ExternalOutput")

    with tile.TileContext(nc) as tc, ExitStack() as ctx:
        const_p = ctx.enter_context(tc.tile_pool(name="const", bufs=1))
        h_p = ctx.enter_context(tc.tile_pool(name="h", bufs=2))
        e_p = ctx.enter_context(tc.tile_pool(name="E", bufs=2))
        ht_p = ctx.enter_context(tc.tile_pool(name="hT", bufs=2))
        tr_p = ctx.enter_context(tc.tile_pool(name="tr", bufs=1))
        s_p = ctx.enter_context(tc.tile_pool(name="S", bufs=2))
        o_p = ctx.enter_context(tc.tile_pool(name="o", bufs=2))
        scr_p = ctx.enter_context(tc.tile_pool(name="scr", bufs=2))
        psT_p = ctx.enter_context(tc.tile_pool(name="psT", bufs=2, space="PSUM"))
        psS_p = ctx.enter_context(tc.tile_pool(name="psS", bufs=2, space="PSUM"))
        psO_p = ctx.enter_context(tc.tile_pool(name="psO", bufs=2, space="PSUM"))

        tblk = const_p.tile([1, 4], F32)
        nc.sync.dma_start(tblk[:], tbl_d.ap())
        ident = const_p.tile([P, P], BF16)
        nc.sync.dma_start(ident[:], ident_d.ap())
        ones = const_p.tile([P, 1], BF16)
        nc.sync.dma_start(ones[:], ones_d.ap())
        wf = const_p.tile([P, 2, D], F32)
        nc.sync.dma_start(wf[:, 0, :], w_d.ap()[0:P, :])
        nc.sync.dma_start(wf[:, 1, :], w_d.ap()[P:2 * P, :])
        wb = const_p.tile([P, 2, D], BF16)
        nc.vector.tensor_copy(wb[:], wf[:])

        def issue_load(c):
            bs = c * P
            t = h_p.tile([P, N_BLOCK, D], BF16)
            # chunk 0's first quarter arrives as 4n slivers so the PE can
            # start transposing ~5us earlier during the pipeline fill
            gsz = 4 if c == 0 else 16
            for g in range(N_BLOCK // gsz):
                nc.gpsimd.dma_start(
                    t[:, g * gsz:(g + 1) * gsz, :],
                    h_d.ap()[bs:bs + P, g * gsz:(g + 1) * gsz, :],
                )
                if c == 0 and g == 3:
                    gsz = 16
                    # remaining 48 n in three 16n loads
                    for g2 in range(1, 4):
                        nc.gpsimd.dma_start(
                            t[:, g2 * 16:(g2 + 1) * 16, :],
                            h_d.ap()[bs:bs + P, g2 * 16:(g2 + 1) * 16, :],
                        )
                    break
            return t

        def emit_sigma_ng(prev, ngF):
            """Sigma_e over e for n-group ngF of the PREVIOUS chunk: PE
            re-transposes F per (n, e-half), a copy moves the slab to SBUF,
            then per n two 1-moving-row matmuls contract the 128 e-partitions
            against the ones vector into psO[:, n]."""
            E_prev, psO = prev["E"], prev["psO"]
            FT = ht_p.tile([P, 16, P], BF16, tag="", name="FT")
            for half in range(2):
                psFT = psT_p.tile([P, 8, P], BF16, tag="psT", name="psFT")
                for j in range(4):
                    n = ngF * 8 + half * 4 + j
                    for eh in range(2):
                        nc.tensor.transpose(
                            psFT[:, 2 * j + eh, :],
                            E_prev[:, n, eh * P:(eh + 1) * P],
                            ident[:],
                        )
                dst = FT[:, half * 8:half * 8 + 8, :].bitcast(F32)
                if ngF % 2 == 0:
                    nc.scalar.copy(dst, psFT[:].bitcast(F32))
                else:
                    nc.vector.tensor_copy(dst, psFT[:].bitcast(F32))
            for j in range(8):
                n = ngF * 8 + j
                nc.tensor.matmul(
                    psO[:, n:n + 1], FT[:, 2 * j, :], ones[:],
                    start=True, stop=False)
                nc.tensor.matmul(
                    psO[:, n:n + 1], FT[:, 2 * j + 1, :], ones[:],
                    start=False, stop=True)

        def finish_prev(prev):
            staged = o_p.tile([P, N_BLOCK], F32, tag="staged")
            nc.scalar.copy(staged[:], prev["psO"][:])
            nc.sync.dma_start(
                out_d.ap()[prev["bs"]:prev["bs"] + P, :], staged[:])

        def emit_act_reduce(prev, k):
            """Final Sigma_e for n 36+7k..36+7k+6 of the PREVIOUS chunk on
            the ACT engine: Copy with accum_out sums the 256 e-elements per
            partition. ACT coexists with DVE at full rate (unlike Pool), and
            emitting inside the next chunk's ng loop avoids head-of-line
            blocking the ACT queue behind the previous chunk's DVE tail."""
            E_prev, staged_prev = prev["E"], prev["staged"]
            scr = scr_p.tile([P, D], BF16, name="scr")
            n0 = prev["act_n0"] + 7 * k
            for n in range(n0, min(n0 + 7, N_BLOCK)):
                nc.scalar.activation(scr[:], E_prev[:, n, :], ACT_T.Copy,
                                     accum_out=staged_prev[:, n:n + 1])

        prev = None
        h_tiles = {0: issue_load(0)}
        for c in range(N_CHUNKS):
            bs = c * P
            # pre-issue next chunk's load ahead of this chunk's Pool work
            if c + 1 < N_CHUNKS:
                h_tiles[c + 1] = issue_load(c + 1)
            h_nat = h_tiles.pop(c)
            E = e_p.tile([P, N_BLOCK, D], BF16)
            sqs = []  # per-quarter n-tree partial sums
            psS_prev = None  # (psS tile, n0) pending exp
            for ng in range(8):  # n-groups of 8
                hT = ht_p.tile([P, 16, P], BF16)
                for half in range(2):
                    psT = psT_p.tile([P, 8, P], BF16, tag="psT")
                    for j in range(4):
                        n = ng * 8 + half * 4 + j
                        for dh in range(2):
                            nc.tensor.transpose(
                                psT[:, 2 * j + dh, :],
                                h_nat[:, n, dh * P:(dh + 1) * P],
                                ident[:],
                            )
                    # packed f32-bitcast copy: half the elements of a bf16
                    # copy (f32->f32 Copy is a bit-exact identity for
                    # normals; bf16 pairs never form denormal/NaN f32
                    # patterns since the high bf16 is a finite normal)
                    nc.scalar.copy(
                        hT[:, half * 8:half * 8 + 8, :].bitcast(F32),
                        psT[:].bitcast(F32))
                # lagged exp from the previous n-group keeps the ACT queue
                # ordered copy(ng) -> exp(ng-1,q1) so PE never waits on hT
                if psS_prev is not None:
                    ps, pn0 = psS_prev
                    nc.scalar.activation(E[:, pn0:pn0 + 4, :], ps[:], ACT_T.Exp)
                for q in range(2):
                    psS = psS_p.tile([P, 4, D], F32)
                    for j in range(4):
                        jj = q * 4 + j
                        nc.tensor.matmul(
                            psS[:, j, :], hT[:, 2 * jj, :], wb[:, 0, :],
                            start=True, stop=False,
                        )
                        nc.tensor.matmul(
                            psS[:, j, :], hT[:, 2 * jj + 1, :], wb[:, 1, :],
                            start=False, stop=True,
                        )
                    if q == 0:
                        nc.scalar.activation(
                            E[:, ng * 8:ng * 8 + 4, :], psS[:], ACT_T.Exp)
                    else:
                        psS_prev = (psS, ng * 8 + 4)
                if prev is not None and 3 <= ng <= 6:
                    emit_act_reduce(prev, ng - 3)
                if ng == 4:
                    # half 0 of E is complete: start its n-tree on DVE and
                    # the in-place P1 = E*h (after the tree's only E-read,
                    # level 1)
                    sh0 = _half_tree(nc.vector, E, tr_p, s_p, 0)
                    nc.vector.tensor_tensor(
                        E[:, 0:32, :], E[:, 0:32, :], h_nat[:, 0:32, :],
                        ALU.mult)
            ps, pn0 = psS_prev
            nc.scalar.activation(E[:, pn0:pn0 + 4, :], ps[:], ACT_T.Exp)
            if prev is not None:
                nc.sync.dma_start(
                    out_d.ap()[prev["bs"]:prev["bs"] + P, :],
                    prev["staged"][:])
            sh1 = _half_tree(nc.vector, E, tr_p, s_p, 1)
            nc.vector.tensor_tensor(
                E[:, 32:64, :], E[:, 32:64, :], h_nat[:, 32:64, :], ALU.mult)

            S = s_p.tile([P, D], F32, tag="S")
            nc.vector.tensor_tensor(S[:], sh0[:], sh1[:], ALU.add)
            R = s_p.tile([P, D], F32, tag="R")
            nc.vector.reciprocal_approx_fast(R[:], S[:])
            Rb = s_p.tile([P, D], BF16, tag="Rb")
            nc.vector.tensor_copy(Rb[:], R[:])

            # F = P1 * R (R broadcast over n) in place on DVE. Half-size
            # instructions keep the 2x bf16 mode (the full-size variant
            # measured at ~1x).
            rb_b = Rb[:].unsqueeze(1).broadcast_to((P, 32, D))
            staged = o_p.tile([P, N_BLOCK], F32, tag="staged")
            # Sigma_e: full-width tensor_reduce per n-half on DVE, right
            # after that half's F so the first reduce overlaps the second
            # multiply. Pool stays off the hot path entirely (concurrent
            # Pool streaming inflates DVE instruction time ~2.5x via SBUF
            # contention) and the PE-based reduction variants lose to this
            # on measured wall-clock.
            nc.vector.tensor_tensor(E[:, 0:32, :], E[:, 0:32, :], rb_b,
                                    ALU.mult)
            nc.vector.tensor_reduce(
                staged[:, 0:32], E[:, 0:32, :], mybir.AxisListType.X,
                ALU.add)
            nc.vector.tensor_tensor(E[:, 32:64, :], E[:, 32:64, :], rb_b,
                                    ALU.mult)
            act_n0 = 40
            nc.vector.tensor_reduce(
                staged[:, 32:act_n0], E[:, 32:act_n0, :],
                mybir.AxisListType.X, ALU.add)
            prev = {"E": E, "staged": staged, "bs": bs, "act_n0": act_n0}
        # epilogue: the last chunk's ACT reduces run concurrently with its
        # DVE tensor_reduce tail, then the store
        for k in range(4):
            emit_act_reduce(prev, k)
        nc.sync.dma_start(out_d.ap()[prev["bs"]:prev["bs"] + P, :],
                          prev["staged"][:])
    nc.compile()
    return nc


def build_bias(tbl_hash):
    """Bias path (kept from the known-good baseline; b_a is zeros in the
    reference setup so this is correctness insurance only)."""
    nc = bacc.Bacc("TRN2", debug=False, num_devices=N_CORES)
    tbl_d = nc.dram_tensor(f"tblkey_{tbl_hash}", [1, 4], F32, kind="ExternalInput")
    h_d = nc.dram_tensor("h", [B_SHARD, N_BLOCK, D], F32, kind="ExternalInput")
    w_d = nc.dram_tensor("W_a", [D, D], F32, kind="ExternalInput")
    ident_d = nc.dram_tensor("ident", [P, P], BF16, kind="ExternalInput")
    ba_d = nc.dram_tensor("b_a", [N_BLOCK, D], F32, kind="ExternalInput")
    out_d = nc.dram_tensor("out", [B_SHARD, N_BLOCK], F32, kind="ExternalOutput")

    with tile.TileContext(nc) as tc, ExitStack() as ctx:
        const_p = ctx.enter_context(tc.tile_pool(name="const", bufs=1))
        h_p = ctx.enter_context(tc.tile_pool(name="h", bufs=2))
        e_p = ctx.enter_context(tc.tile_pool(name="E", bufs=2))
        ht_p = ctx.enter_context(tc.tile_pool(name="hT", bufs=2))
        t_p = ctx.enter_context(tc.tile_pool(name="t", bufs=2))
        tr_p = ctx.enter_context(tc.tile_pool(name="tr", bufs=1))
        s_p = ctx.enter_context(tc.tile_pool(name="S", bufs=2))
        scr_p = ctx.enter_context(tc.tile_pool(name="scr", bufs=4))
        o_p = ctx.enter_context(tc.tile_pool(name="o", bufs=2))
        psT_p = ctx.enter_context(tc.tile_pool(name="psT", bufs=2, space="PSUM"))
        psS_p = ctx.enter_context(tc.tile_pool(name="psS", bufs=2, space="PSUM"))

        tblk = const_p.tile([1, 4], F32)
        nc.sync.dma_start(tblk[:], tbl_d.ap())
        ident = const_p.tile([P, P], BF16)
        nc.sync.dma_start(ident[:], ident_d.ap())
        wf = const_p.tile([P, 2, D], F32)
        nc.sync.dma_start(wf[:, 0, :], w_d.ap()[0:P, :])
        nc.sync.dma_start(wf[:, 1, :], w_d.ap()[P:2 * P, :])
        wb = const_p.tile([P, 2, D], BF16)
        nc.vector.tensor_copy(wb[:], wf[:])
        bab = const_p.tile([P, N_BLOCK, D], BF16)
        src = ba_d.ap().rearrange("(one n) d -> one n d", one=1)
        src = src.broadcast_to((P, N_BLOCK, D))
        nc.gpsimd.dma_start(bab[:], src)

        for c in range(N_CHUNKS):
            bs = c * P
            h_nat = h_p.tile([P, N_BLOCK, D], BF16)
            for g in range(4):
                nc.gpsimd.dma_start(
                    h_nat[:, g * 16:(g + 1) * 16, :],
                    h_d.ap()[bs:bs + P, g * 16:(g + 1) * 16, :],
                )
            E = e_p.tile([P, N_BLOCK, D], BF16)
            for ng in range(8):
                psT = psT_p.tile([P, 16, P], BF16)
                hT = ht_p.tile([P, 16, P], BF16)
                for j in range(8):
                    n = ng * 8 + j
                    for dh in range(2):
                        nc.tensor.transpose(
                            psT[:, 2 * j + dh, :],
                            h_nat[:, n, dh * P:(dh + 1) * P],
                            ident[:],
                        )
                nc.scalar.copy(hT[:], psT[:])
                for q in range(2):
                    psS = psS_p.tile([P, 4, D], F32)
                    for j in range(4):
                        jj = q * 4 + j
                        nc.tensor.matmul(
                            psS[:, j, :], hT[:, 2 * jj, :], wb[:, 0, :],
                            start=True, stop=False,
                        )
                        nc.tensor.matmul(
                            psS[:, j, :], hT[:, 2 * jj + 1, :], wb[:, 1, :],
                            start=False, stop=True,
                        )
                    n0 = ng * 8 + q * 4
                    tb = t_p.tile([P, 4, D], F32, tag="tbias")
                    nc.vector.tensor_add(tb[:], psS[:], bab[:, n0:n0 + 4, :])
                    nc.scalar.activation(E[:, n0:n0 + 4, :], tb[:], ACT_T.Exp)

            s_half = []
            for half in range(2):
                n0 = half * 32
                tr1 = tr_p.tile([P, 16, D], BF16, tag=f"tr1_{half}")
                tr2 = tr_p.tile([P, 8, D], BF16, tag=f"tr2_{half}")
                nc.vector.tensor_tensor(
                    tr1[:], E[:, n0:n0 + 16, :], E[:, n0 + 16:n0 + 32, :], ALU.add)
                nc.vector.tensor_tensor(
                    tr2[:], tr1[:, 0:8, :], tr1[:, 8:16, :], ALU.add)
                nc.vector.tensor_tensor(
                    tr1[:, 0:4, :], tr2[:, 0:4, :], tr2[:, 4:8, :], ALU.add)
                nc.vector.tensor_tensor(
                    tr2[:, 0:2, :], tr1[:, 0:2, :], tr1[:, 2:4, :], ALU.add)
                sh = s_p.tile([P, D], F32, tag=f"Sh{half}")
                nc.vector.tensor_tensor(sh[:], tr2[:, 0, :], tr2[:, 1, :], ALU.add)
                s_half.append(sh)
                nc.vector.tensor_tensor(
                    E[:, n0:n0 + 32, :], E[:, n0:n0 + 32, :],
                    h_nat[:, n0:n0 + 32, :], ALU.mult
                )
            S = s_p.tile([P, D], F32, tag="S")
            nc.vector.tensor_tensor(S[:], s_half[0][:], s_half[1][:], ALU.add)
            R = s_p.tile([P, D], F32, tag="R")
            nc.vector.reciprocal(R[:], S[:])
            Rb = s_p.tile([P, D], BF16, tag="Rb")
            nc.vector.tensor_copy(Rb[:], R[:])

            out_sb = o_p.tile([P, N_BLOCK], F32, tag="out_sb")
            for n in range(N_BLOCK):
                scr = scr_p.tile([P, D], BF16)
                nc.vector.scalar_tensor_tensor(
                    out=scr[:],
                    in0=E[:, n, :],
                    scalar=1.0,
                    in1=Rb[:],
                    op0=ALU.mult,
                    op1=ALU.mult,
                    accum_out=out_sb[:, n:n + 1],
                )
            staged = o_p.tile([P, N_BLOCK], F32, tag="staged")
            nc.vector.tensor_copy(staged[:], out_sb[:])
            nc.gpsimd.dma_start(out_d.ap()[bs:bs + P, :], staged[:])
    nc.compile()
    return nc


_CACHE = {}


def _get_nc(with_bias: bool):
    if with_bias not in _CACHE:
        tbl_hash = _install_act_tables()
        _CACHE[with_bias] = (
            build_bias(tbl_hash) if with_bias else build_fast(tbl_hash))
    return _CACHE[with_bias]


def run(h, W_a, b_a, trace=False):
    import ml_dtypes

    tbl_hash = _install_act_tables()

    h = np.ascontiguousarray(np.asarray(h, dtype=np.float32))
    W_a = np.ascontiguousarray(np.asarray(W_a, dtype=np.float32))
    b_a = np.ascontiguousarray(np.asarray(b_a, dtype=np.float32))
    with_bias = bool(np.any(b_a))
    nc = _get_nc(with_bias)
    ident = np.eye(P, dtype=ml_dtypes.bfloat16)
    in_maps = []
    for i in range(N_CORES):
        m = {
            "h": h[i * B_SHARD: (i + 1) * B_SHARD],
            "W_a": W_a,
            "ident": ident,
            f"tblkey_{tbl_hash}": np.zeros((1, 4), np.float32),
        }
        if with_bias:
            m["b_a"] = b_a
        else:
            m["ones"] = np.ones((P, 1), dtype=ml_dtypes.bfloat16)
        in_maps.append(m)
    res = run_bass_kernel_spmd(nc, in_maps, core_ids=list(range(N_CORES)), trace=trace)
    out = np.concatenate([res.results[i]["out"] for i in range(N_CORES)], axis=0)
    return out, res


def kernel(h, W_a, b_a):
    out, _ = run(h, W_a, b_a, trace=False)
    return out


# revision 42
# speedup vs baseline: 1.2154x; 1.0282x over previous
"""AspectAttention Trainium2 kernel (8 NeuronCores, pure data parallel).

out[b, n] = sum_e softmax_n(tanh(h @ W_a + b_a))[b, n, e] * h[b, n, e]

Self-contained: hardcodes shapes B=4096, N=64, D=256, 8 cores.

Per-core dataflow (512 batches), batch-on-partitions layout:
  - SWDGE cast-DMA on the sync queue: h f32 DRAM -> bf16 SBUF chunks
    [128 b, 64 n, 256 d] (contiguous per partition -> full-rate)
  - PE transpose per (n, d-half): [128 b, 128 d] -> PSUM [128 d, 128 b] bf16
  - ACT copies PSUM -> SBUF hT slabs as packed uint32 (half the elems),
    queued ahead of exp so the PE never waits on hT
  - PE matmul: stationary hT-slab [d, b], moving W [d, e] -> PSUM s[b, e],
    accumulated over both d-halves
  - ACT: custom LUT where `Exp` evaluates exp(tanh(x)) (PSUM->SBUF bf16);
    tanh in [-1,1] bounds the softmax domain, so no max-subtraction.
  - Pool: tree-sum over n -> S[b, e] (f32); DVE reciprocal -> R, cast Rb
  - DVE: P1 = E*h in place; F = P1*Rb (R broadcast over n, split DVE/Pool)
  - Sigma_e: in-place halving tree over e (bf16 2x TT, split DVE/Pool)
    then one tensor_reduce(axis=X) on the last 32 -> out[b, n] f32
"""
import hashlib
import json
import os
import shutil
from contextlib import ExitStack

import numpy as np

_PWP_SRC = (
    "/nix/store/z022hj2nvbm3nwdizlisq4ylc0y7rd6q-python3-3.13.14-env/"
    "lib/python3.13/site-packages/neuronxcc/pwp/pwp_bin_trainium"
)


def _exptanh_derivs(x):
    u = np.tanh(x)
    s = 1.0 - u * u
    f = np.exp(u)
    return (f, f * s, f * (s * s - 2 * u * s),
            f * (s**3 - 6 * u * s * s - 2 * s * s + 4 * u * u * s))


def _install_act_tables():
    """Build ACT tables where func `exp` evaluates exp(tanh(x)). Returns a
    content hash so the compile cache keys on the table contents."""
    global _PWP_SRC
    if not os.path.isdir(_PWP_SRC):
        from neuronxcc.driver.Job import Job
        from neuronxcc.driver.jobs.support.FindActInfo import findActInfoFile
        _PWP_SRC = os.path.dirname(findActInfoFile(Job.getPackageDir(), "gen3"))
    dst = "/tmp/aspect_act_tables_v1"
    if not os.path.exists(os.path.join(dst, "act_info.json")):
        tmp = dst + ".tmp"
        if os.path.exists(tmp):
            shutil.rmtree(tmp)
        shutil.copytree(_PWP_SRC, tmp)
        bkt_path = os.path.join(tmp, "exp_and_others_bkt.bin")
        b = np.fromfile(bkt_path, dtype=np.float32).reshape(-1, 8).copy()
        x0 = b[:, 4].astype(np.float64)
        d0, d1, d2 = b[:, 0], b[:, 1], b[:, 2]
        with np.errstate(over="ignore", invalid="ignore"):
            ex = np.exp(np.clip(x0, -87.0, 87.0))
            is_exp = (np.isfinite(d0)
                      & (np.abs(d0 - ex) <= 1e-3 * np.maximum(ex, 1e-30))
                      & (np.abs(d1 - d0) <= 1e-3 * np.abs(d0) + 1e-30)
                      & (np.abs(d2 - d0 / 2) <= 1e-3 * np.abs(d0) + 1e-30))
        idx = np.where(is_exp)[0]
        f, f1, f2, f3 = _exptanh_derivs(x0[idx])
        b[idx, 0] = f.astype(np.float32)
        b[idx, 1] = f1.astype(np.float32)
        b[idx, 2] = (f2 / 2.0).astype(np.float32)
        b[idx, 3] = (f3 / 6.0).astype(np.float32)
        b[779] = [np.float32(np.e), 0, 0, 0, 0, 0, 0, 0]
        b[780] = [np.float32(1 / np.e), 0, 0, 0, 0, 0, 0, 0]
        b.tofile(bkt_path)
        pj_path = os.path.join(tmp, "exp_and_others.json")
        pj = json.load(open(pj_path))
        for fm in pj["profile_meta_data"]:
            if fm["func_name"].startswith("exp"):
                fm["fpinf_result"] = int(np.float32(np.e).view(np.uint32))
                fm["fninf_result"] = int(np.float32(1 / np.e).view(np.uint32))
        json.dump(pj, open(pj_path, "w"))
        os.replace(tmp, dst) if not os.path.exists(dst) else None
    os.environ["BASS_ACT_ROOT_JSON_PATH"] = os.path.join(dst, "act_info.json")
    hsh = hashlib.sha256(
        open(os.path.join(dst, "exp_and_others_bkt.bin"), "rb").read()
    ).hexdigest()[:8]
    return hsh

import concourse.bass as bass
import concourse.tile as tile
from concourse import bacc, mybir
from concourse.bass_utils import run_bass_kernel_spmd

N_CORES = 8
B_FULL, N_BLOCK, D = 4096, 64, 256
B_SHARD = B_FULL // N_CORES  # 512
P = 128
N_CHUNKS = B_SHARD // P  # 4
F32 = mybir.dt.float32
BF16 = mybir.dt.bfloat16
U32 = mybir.dt.uint32
ALU = mybir.AluOpType
ACT_T = mybir.ActivationFunctionType


def _quarter_tree(eng, E, tr_p, s_p, q):
    """Sum E[:, q*16:(q+1)*16, :] over n with a TT add tree on `eng`.
    Quarter granularity starts the DVE two n-groups earlier than halves and
    shortens the post-exp tail chain."""
    a = q * 16
    t1 = tr_p.tile([P, 8, D], BF16, tag=f"tq1_{q}")
    t2 = tr_p.tile([P, 4, D], BF16, tag=f"tq2_{q}")
    eng.tensor_tensor(t1[:], E[:, a:a + 8, :], E[:, a + 8:a + 16, :], ALU.add)
    eng.tensor_tensor(t2[:], t1[:, 0:4, :], t1[:, 4:8, :], ALU.add)
    eng.tensor_tensor(t1[:, 0:2, :], t2[:, 0:2, :], t2[:, 2:4, :], ALU.add)
    sq = s_p.tile([P, D], F32, tag=f"Sq{q}")
    eng.tensor_tensor(sq[:], t1[:, 0, :], t1[:, 1, :], ALU.add)
    return sq


def _half_tree(eng, E, tr_p, s_p, half):
    """Sum E[:, half*32:(half+1)*32, :] over n with a TT add tree on `eng`.
    Returns the [P, D] f32 partial sum tile."""
    n0 = half * 32
    tr1 = tr_p.tile([P, 16, D], BF16, tag=f"tr1_{half}")
    tr2 = tr_p.tile([P, 8, D], BF16, tag=f"tr2_{half}")
    eng.tensor_tensor(tr1[:], E[:, n0:n0 + 16, :], E[:, n0 + 16:n0 + 32, :],
                      ALU.add)
    eng.tensor_tensor(tr2[:], tr1[:, 0:8, :], tr1[:, 8:16, :], ALU.add)
    eng.tensor_tensor(tr1[:, 0:4, :], tr2[:, 0:4, :], tr2[:, 4:8, :], ALU.add)
    eng.tensor_tensor(tr2[:, 0:2, :], tr1[:, 0:2, :], tr1[:, 2:4, :], ALU.add)
    sh = s_p.tile([P, D], F32, tag=f"Sh{half}")
    eng.tensor_tensor(sh[:], tr2[:, 0, :], tr2[:, 1, :], ALU.add)
    return sh


def build_fast(tbl_hash):
    """Optimized no-bias path."""
    nc = bacc.Bacc("TRN2", debug=False, num_devices=N_CORES)
    tbl_d = nc.dram_tensor(f"tblkey_{tbl_hash}", [1, 4], F32, kind="ExternalInput")
    h_d = nc.dram_tensor("h", [B_SHARD, N_BLOCK, D], F32, kind="ExternalInput")
    w_d = nc.dram_tensor("W_a", [D, D], F32, kind="ExternalInput")
    ident_d = nc.dram_tensor("ident", [P, P], BF16, kind="ExternalInput")
    ones_d = nc.dram_tensor("ones", [P, 1], BF16, kind="ExternalInput")
    out_d = nc.dram_tensor("out", [B_SHARD, N_BLOCK], F32, kind="ExternalOutput")

    with tile.TileContext(nc) as tc, ExitStack() as ctx:
        const_p = ctx.enter_context(tc.tile_pool(name="const", bufs=1))
        h_p = ctx.enter_context(tc.tile_pool(name="h", bufs=2))
        e_p = ctx.enter_context(tc.tile_pool(name="E", bufs=2))
        ht_p = ctx.enter_context(tc.tile_pool(name="hT", bufs=2))
        tr_p = ctx.enter_context(tc.tile_pool(name="tr", bufs=1))
        s_p = ctx.enter_context(tc.tile_pool(name="S", bufs=2))
        o_p = ctx.enter_context(tc.tile_pool(name="o", bufs=2))
        scr_p = ctx.enter_context(tc.tile_pool(name="scr", bufs=2))
        psT_p = ctx.enter_context(tc.tile_pool(name="psT", bufs=2, space="PSUM"))
        psS_p = ctx.enter_context(tc.tile_pool(name="psS", bufs=2, space="PSUM"))
        psO_p = ctx.enter_context(tc.tile_pool(name="psO", bufs=2, space="PSUM"))

        tblk = const_p.tile([1, 4], F32)
        nc.sync.dma_start(tblk[:], tbl_d.ap())
        ident = const_p.tile([P, P], BF16)
        nc.sync.dma_start(ident[:], ident_d.ap())
        ones = const_p.tile([P, 1], BF16)
        nc.sync.dma_start(ones[:], ones_d.ap())
        wf = const_p.tile([P, 2, D], F32)
        nc.sync.dma_start(wf[:, 0, :], w_d.ap()[0:P, :])
        nc.sync.dma_start(wf[:, 1, :], w_d.ap()[P:2 * P, :])
        wb = const_p.tile([P, 2, D], BF16)
        nc.vector.tensor_copy(wb[:], wf[:])

        def issue_load(c):
            bs = c * P
            t = h_p.tile([P, N_BLOCK, D], BF16)
            # chunk 0's first quarter arrives as 4n slivers so the PE can
            # start transposing ~5us earlier during the pipeline fill
            gsz = 4 if c == 0 else 16
            for g in range(N_BLOCK // gsz):
                nc.gpsimd.dma_start(
                    t[:, g * gsz:(g + 1) * gsz, :],
                    h_d.ap()[bs:bs + P, g * gsz:(g + 1) * gsz, :],
                )
                if c == 0 and g == 3:
                    gsz = 16
                    # remaining 48 n in three 16n loads
                    for g2 in range(1, 4):
                        nc.gpsimd.dma_start(
                            t[:, g2 * 16:(g2 + 1) * 16, :],
                            h_d.ap()[bs:bs + P, g2 * 16:(g2 + 1) * 16, :],
                        )
                    break
            return t

        def emit_sigma_ng(prev, ngF):
            """Sigma_e over e for n-group ngF of the PREVIOUS chunk: PE
            re-transposes F per (n, e-half), a copy moves the slab to SBUF,
            then per n two 1-moving-row matmuls contract the 128 e-partitions
            against the ones vector into psO[:, n]."""
            E_prev, psO = prev["E"], prev["psO"]
            FT = ht_p.tile([P, 16, P], BF16, tag="", name="FT")
            for half in range(2):
                psFT = psT_p.tile([P, 8, P], BF16, tag="psT", name="psFT")
                for j in range(4):
                    n = ngF * 8 + half * 4 + j
                    for eh in range(2):
                        nc.tensor.transpose(
                            psFT[:, 2 * j + eh, :],
                            E_prev[:, n, eh * P:(eh + 1) * P],
                            ident[:],
                        )
                dst = FT[:, half * 8:half * 8 + 8, :].bitcast(F32)
                if ngF % 2 == 0:
                    nc.scalar.copy(dst, psFT[:].bitcast(F32))
                else:
                    nc.vector.tensor_copy(dst, psFT[:].bitcast(F32))
            for j in range(8):
                n = ngF * 8 + j
                nc.tensor.matmul(
                    psO[:, n:n + 1], FT[:, 2 * j, :], ones[:],
                    start=True, stop=False)
                nc.tensor.matmul(
                    psO[:, n:n + 1], FT[:, 2 * j + 1, :], ones[:],
                    start=False, stop=True)

        def finish_prev(prev):
            staged = o_p.tile([P, N_BLOCK], F32, tag="staged")
            nc.scalar.copy(staged[:], prev["psO"][:])
            nc.sync.dma_start(
                out_d.ap()[prev["bs"]:prev["bs"] + P, :], staged[:])

        def emit_act_reduce(prev, k):
            """Final Sigma_e for n 36+7k..36+7k+6 of the PREVIOUS chunk on
            the ACT engine: Copy with accum_out sums the 256 e-elements per
            partition. ACT coexists with DVE at full rate (unlike Pool), and
            emitting inside the next chunk's ng loop avoids head-of-line
            blocking the ACT queue behind the previous chunk's DVE tail."""
            E_prev, staged_prev = prev["E"], prev["staged"]
            scr = scr_p.tile([P, D], BF16, name="scr")
            n0 = prev["act_n0"] + 7 * k
            for n in range(n0, min(n0 + 7, N_BLOCK)):
                nc.scalar.activation(scr[:], E_prev[:, n, :], ACT_T.Copy,
                                     accum_out=staged_prev[:, n:n + 1])

        # warm the PE pstate with dummy ident transposes during the initial
        # h load (PE would otherwise idle cold and start at 0.65-1.2 GHz)
        warm = psT_p.tile([P, 8, P], BF16, tag="psT", name="warm")
        for i in range(24):
            nc.tensor.transpose(warm[:, i % 8, :], ident[:], ident[:])

        prev = None
        h_tiles = {0: issue_load(0)}
        for c in range(N_CHUNKS):
            bs = c * P
            # pre-issue next chunk's load ahead of this chunk's Pool work
            if c + 1 < N_CHUNKS:
                h_tiles[c + 1] = issue_load(c + 1)
            h_nat = h_tiles.pop(c)
            E = e_p.tile([P, N_BLOCK, D], BF16)
            sqs = []  # per-quarter n-tree partial sums
            psS_prev = None  # (psS tile, n0) pending exp
            for ng in range(8):  # n-groups of 8
                hT = ht_p.tile([P, 16, P], BF16)
                for half in range(2):
                    psT = psT_p.tile([P, 8, P], BF16, tag="psT")
                    for j in range(4):
                        n = ng * 8 + half * 4 + j
                        for dh in range(2):
                            nc.tensor.transpose(
                                psT[:, 2 * j + dh, :],
                                h_nat[:, n, dh * P:(dh + 1) * P],
                                ident[:],
                            )
                    # packed f32-bitcast copy: half the elements of a bf16
                    # copy (f32->f32 Copy is a bit-exact identity for
                    # normals; bf16 pairs never form denormal/NaN f32
                    # patterns since the high bf16 is a finite normal)
                    nc.scalar.copy(
                        hT[:, half * 8:half * 8 + 8, :].bitcast(F32),
                        psT[:].bitcast(F32))
                # lagged exp from the previous n-group keeps the ACT queue
                # ordered copy(ng) -> exp(ng-1,q1) so PE never waits on hT
                if psS_prev is not None:
                    ps, pn0 = psS_prev
                    nc.scalar.activation(E[:, pn0:pn0 + 4, :], ps[:], ACT_T.Exp)
                for q in range(2):
                    psS = psS_p.tile([P, 4, D], F32)
                    for j in range(4):
                        jj = q * 4 + j
                        nc.tensor.matmul(
                            psS[:, j, :], hT[:, 2 * jj, :], wb[:, 0, :],
                            start=True, stop=False,
                        )
                        nc.tensor.matmul(
                            psS[:, j, :], hT[:, 2 * jj + 1, :], wb[:, 1, :],
                            start=False, stop=True,
                        )
                    if q == 0:
                        nc.scalar.activation(
                            E[:, ng * 8:ng * 8 + 4, :], psS[:], ACT_T.Exp)
                    else:
                        psS_prev = (psS, ng * 8 + 4)
                if prev is not None and 3 <= ng <= 6:
                    emit_act_reduce(prev, ng - 3)
                if ng in (2, 4, 6):
                    # quarter q of E is complete: its n-tree, then in-place
                    # P1 = E*h (after the tree's only E-read, level 1)
                    q = ng // 2 - 1
                    sqs.append(_quarter_tree(nc.vector, E, tr_p, s_p, q))
                    nc.vector.tensor_tensor(
                        E[:, q * 16:(q + 1) * 16, :],
                        E[:, q * 16:(q + 1) * 16, :],
                        h_nat[:, q * 16:(q + 1) * 16, :], ALU.mult)
            ps, pn0 = psS_prev
            nc.scalar.activation(E[:, pn0:pn0 + 4, :], ps[:], ACT_T.Exp)
            if prev is not None:
                nc.sync.dma_start(
                    out_d.ap()[prev["bs"]:prev["bs"] + P, :],
                    prev["staged"][:])
            sqs.append(_quarter_tree(nc.vector, E, tr_p, s_p, 3))
            nc.vector.tensor_tensor(
                E[:, 48:64, :], E[:, 48:64, :], h_nat[:, 48:64, :], ALU.mult)

            s01 = s_p.tile([P, D], F32, tag="s01")
            nc.vector.tensor_tensor(s01[:], sqs[0][:], sqs[1][:], ALU.add)
            S = s_p.tile([P, D], F32, tag="S")
            nc.vector.tensor_tensor(S[:], sqs[2][:], sqs[3][:], ALU.add)
            nc.vector.tensor_tensor(S[:], S[:], s01[:], ALU.add)
            R = s_p.tile([P, D], F32, tag="R")
            nc.vector.reciprocal_approx_fast(R[:], S[:])
            Rb = s_p.tile([P, D], BF16, tag="Rb")
            nc.vector.tensor_copy(Rb[:], R[:])

            # F = P1 * R (R broadcast over n) in place on DVE. Half-size
            # instructions keep the 2x bf16 mode (the full-size variant
            # measured at ~1x).
            rb_b = Rb[:].unsqueeze(1).broadcast_to((P, 32, D))
            staged = o_p.tile([P, N_BLOCK], F32, tag="staged")
            # Sigma_e: full-width tensor_reduce per n-half on DVE, right
            # after that half's F so the first reduce overlaps the second
            # multiply. Pool stays off the hot path entirely (concurrent
            # Pool streaming inflates DVE instruction time ~2.5x via SBUF
            # contention) and the PE-based reduction variants lose to this
            # on measured wall-clock.
            nc.vector.tensor_tensor(E[:, 0:32, :], E[:, 0:32, :], rb_b,
                                    ALU.mult)
            nc.vector.tensor_reduce(
                staged[:, 0:32], E[:, 0:32, :], mybir.AxisListType.X,
                ALU.add)
            nc.vector.tensor_tensor(E[:, 32:64, :], E[:, 32:64, :], rb_b,
                                    ALU.mult)
            act_n0 = 40
            nc.vector.tensor_reduce(
                staged[:, 32:act_n0], E[:, 32:act_n0, :],
                mybir.AxisListType.X, ALU.add)
            prev = {"E": E, "staged": staged, "bs": bs, "act_n0": act_n0}
        # epilogue: the last chunk's ACT reduces run concurrently with its
        # DVE tensor_reduce tail, then the store
        for k in range(4):
            emit_act_reduce(prev, k)
        nc.sync.dma_start(out_d.ap()[prev["bs"]:prev["bs"] + P, :],
                          prev["staged"][:])
    nc.compile()
    return nc


def build_bias(tbl_hash):
    """Bias path (kept from the known-good baseline; b_a is zeros in the
    reference setup so this is correctness insurance only)."""
    nc = bacc.Bacc("TRN2", debug=False, num_devices=N_CORES)
    tbl_d = nc.dram_tensor(f"tblkey_{tbl_hash}", [1, 4], F32, kind="ExternalInput")
    h_d = nc.dram_tensor("h", [B_SHARD, N_BLOCK, D], F32, kind="ExternalInput")
    w_d = nc.dram_tensor("W_a", [D, D], F32, kind="ExternalInput")
    ident_d = nc.dram_tensor("ident", [P, P], BF16, kind="ExternalInput")
    ba_d = nc.dram_tensor("b_a", [N_BLOCK, D], F32, kind="ExternalInput")
    out_d = nc.dram_tensor("out", [B_SHARD, N_BLOCK], F32, kind="ExternalOutput")

    with tile.TileContext(nc) as tc, ExitStack() as ctx:
        const_p = ctx.enter_context(tc.tile_pool(name="const", bufs=1))
        h_p = ctx.enter_context(tc.tile_pool(name="h", bufs=2))
        e_p = ctx.enter_context(tc.tile_pool(name="E", bufs=2))
        ht_p = ctx.enter_context(tc.tile_pool(name="hT", bufs=2))
        t_p = ctx.enter_context(tc.tile_pool(name="t", bufs=2))
        tr_p = ctx.enter_context(tc.tile_pool(name="tr", bufs=1))
        s_p = ctx.enter_context(tc.tile_pool(name="S", bufs=2))
        scr_p = ctx.enter_context(tc.tile_pool(name="scr", bufs=4))
        o_p = ctx.enter_context(tc.tile_pool(name="o", bufs=2))
        psT_p = ctx.enter_context(tc.tile_pool(name="psT", bufs=2, space="PSUM"))
        psS_p = ctx.enter_context(tc.tile_pool(name="psS", bufs=2, space="PSUM"))

        tblk = const_p.tile([1, 4], F32)
        nc.sync.dma_start(tblk[:], tbl_d.ap())
        ident = const_p.tile([P, P], BF16)
        nc.sync.dma_start(ident[:], ident_d.ap())
        wf = const_p.tile([P, 2, D], F32)
        nc.sync.dma_start(wf[:, 0, :], w_d.ap()[0:P, :])
        nc.sync.dma_start(wf[:, 1, :], w_d.ap()[P:2 * P, :])
        wb = const_p.tile([P, 2, D], BF16)
        nc.vector.tensor_copy(wb[:], wf[:])
        bab = const_p.tile([P, N_BLOCK, D], BF16)
        src = ba_d.ap().rearrange("(one n) d -> one n d", one=1)
        src = src.broadcast_to((P, N_BLOCK, D))
        nc.gpsimd.dma_start(bab[:], src)

        for c in range(N_CHUNKS):
            bs = c * P
            h_nat = h_p.tile([P, N_BLOCK, D], BF16)
            for g in range(4):
                nc.gpsimd.dma_start(
                    h_nat[:, g * 16:(g + 1) * 16, :],
                    h_d.ap()[bs:bs + P, g * 16:(g + 1) * 16, :],
                )
            E = e_p.tile([P, N_BLOCK, D], BF16)
            for ng in range(8):
                psT = psT_p.tile([P, 16, P], BF16)
                hT = ht_p.tile([P, 16, P], BF16)
                for j in range(8):
                    n = ng * 8 + j
                    for dh in range(2):
                        nc.tensor.transpose(
                            psT[:, 2 * j + dh, :],
                            h_nat[:, n, dh * P:(dh + 1) * P],
                            ident[:],
                        )
                nc.scalar.copy(hT[:], psT[:])
                for q in range(2):
                    psS = psS_p.tile([P, 4, D], F32)
                    for j in range(4):
                        jj = q * 4 + j
                        nc.tensor.matmul(
                            psS[:, j, :], hT[:, 2 * jj, :], wb[:, 0, :],
                            start=True, stop=False,
                        )
                        nc.tensor.matmul(
                            psS[:, j, :], hT[:, 2 * jj + 1, :], wb[:, 1, :],
                            start=False, stop=True,
                        )
                    n0 = ng * 8 + q * 4
                    tb = t_p.tile([P, 4, D], F32, tag="tbias")
                    nc.vector.tensor_add(tb[:], psS[:], bab[:, n0:n0 + 4, :])
                    nc.scalar.activation(E[:, n0:n0 + 4, :], tb[:], ACT_T.Exp)

            s_half = []
            for half in range(2):
                n0 = half * 32
                tr1 = tr_p.tile([P, 16, D], BF16, tag=f"tr1_{half}")
                tr2 = tr_p.tile([P, 8, D], BF16, tag=f"tr2_{half}")
                nc.vector.tensor_tensor(
                    tr1[:], E[:, n0:n0 + 16, :], E[:, n0 + 16:n0 + 32, :], ALU.add)
                nc.vector.tensor_tensor(
                    tr2[:], tr1[:, 0:8, :], tr1[:, 8:16, :], ALU.add)
                nc.vector.tensor_tensor(
                    tr1[:, 0:4, :], tr2[:, 0:4, :], tr2[:, 4:8, :], ALU.add)
                nc.vector.tensor_tensor(
                    tr2[:, 0:2, :], tr1[:, 0:2, :], tr1[:, 2:4, :], ALU.add)
                sh = s_p.tile([P, D], F32, tag=f"Sh{half}")
                nc.vector.tensor_tensor(sh[:], tr2[:, 0, :], tr2[:, 1, :], ALU.add)
                s_half.append(sh)
                nc.vector.tensor_tensor(
                    E[:, n0:n0 + 32, :], E[:, n0:n0 + 32, :],
                    h_nat[:, n0:n0 + 32, :], ALU.mult
                )
            S = s_p.tile([P, D], F32, tag="S")
            nc.vector.tensor_tensor(S[:], s_half[0][:], s_half[1][:], ALU.add)
            R = s_p.tile([P, D], F32, tag="R")
            nc.vector.reciprocal(R[:], S[:])
            Rb = s_p.tile([P, D], BF16, tag="Rb")
            nc.vector.tensor_copy(Rb[:], R[:])

            out_sb = o_p.tile([P, N_BLOCK], F32, tag="out_sb")
            for n in range(N_BLOCK):
                scr = scr_p.tile([P, D], BF16)
                nc.vector.scalar_tensor_tensor(
                    out=scr[:],
                    in0=E[:, n, :],
                    scalar=1.0,
                    in1=Rb[:],
                    op0=ALU.mult,
                    op1=ALU.mult,
                    accum_out=out_sb[:, n:n + 1],
                )
            staged = o_p.tile([P, N_BLOCK], F32, tag="staged")
            nc.vector.tensor_copy(staged[:], out_sb[:])
            nc.gpsimd.dma_start(out_d.ap()[bs:bs + P, :], staged[:])
    nc.compile()
    return nc


_CACHE = {}


def _get_nc(with_bias: bool):
    if with_bias not in _CACHE:
        tbl_hash = _install_act_tables()
        _CACHE[with_bias] = (
            build_bias(tbl_hash) if with_bias else build_fast(tbl_hash))
    return _CACHE[with_bias]


def run(h, W_a, b_a, trace=False):
    import ml_dtypes

    tbl_hash = _install_act_tables()

    h = np.ascontiguousarray(np.asarray(h, dtype=np.float32))
    W_a = np.ascontiguousarray(np.asarray(W_a, dtype=np.float32))
    b_a = np.ascontiguousarray(np.asarray(b_a, dtype=np.float32))
    with_bias = bool(np.any(b_a))
    nc = _get_nc(with_bias)
    ident = np.eye(P, dtype=ml_dtypes.bfloat16)
    in_maps = []
    for i in range(N_CORES):
        m = {
            "h": h[i * B_SHARD: (i + 1) * B_SHARD],
            "W_a": W_a,
            "ident": ident,
            f"tblkey_{tbl_hash}": np.zeros((1, 4), np.float32),
        }
        if with_bias:
            m["b_a"] = b_a
        else:
            m["ones"] = np.ones((P, 1), dtype=ml_dtypes.bfloat16)
        in_maps.append(m)
    res = run_bass_kernel_spmd(nc, in_maps, core_ids=list(range(N_CORES)), trace=trace)
    out = np.concatenate([res.results[i]["out"] for i in range(N_CORES)], axis=0)
    return out, res


def kernel(h, W_a, b_a):
    out, _ = run(h, W_a, b_a, trace=False)
    return out
